# revision 1
# baseline (speedup 1.0000x reference)
"""BinaryGPTNeoBlock on 8 trn2 NeuronCores.

Sequence-parallel over 8 cores: core c owns rows {c, c+8, ...} of both
batch elements (256 per batch, 512 total). One 8-core AllGather shares
K/V in bf16 (feature-major K, token-major V); two more share tanh'd MLP
weights (each core tanh's 1/8th). QKV/out-proj/fc matmuls run fp32r
(full PE rate at N>=256); attention and the FF->D projection run bf16.

Self-contained: hardcodes shapes; host only shards/transposes/builds masks.
"""

import numpy as np
import ml_dtypes

import concourse.bass as bass
import concourse.tile as tile
from concourse import bacc, mybir
from concourse.bass_utils import run_bass_kernel_spmd
from concourse.masks import make_identity

B, S, D = 2, 2048, 2048
H = 16
HD = 128
FF = 4 * D
EPS = 1e-5
NC = 8
RPC = S // NC          # 256 rows per core per batch
TL = 2 * RPC           # 512 local rows
NKV = TL * D           # elems of K^T (== of V) per core
WFC_CH = D * FF // NC
WPJ_CH = FF * D // NC

dt = mybir.dt
AF = mybir.ActivationFunctionType
OP = mybir.AluOpType

_CACHE = {}


def _build(apply_g1, apply_b1, apply_g2, apply_b2):
    nc = bacc.Bacc("TRN2", target_bir_lowering=False, debug=False,
                   num_devices=NC)

    xl_d = nc.dram_tensor("xl", [TL, D], dt.float32, kind="ExternalInput").ap()
    wqT_d = nc.dram_tensor("wqT", [D, D], dt.float32, kind="ExternalInput").ap()
    wkT_d = nc.dram_tensor("wkT", [D, D], dt.float32, kind="ExternalInput").ap()
    wvT_d = nc.dram_tensor("wvT", [D, D], dt.float32, kind="ExternalInput").ap()
    woT_d = nc.dram_tensor("woT", [D, D], dt.float32, kind="ExternalInput").ap()
    wfc_ch_d = nc.dram_tensor("wfc_ch", [WFC_CH], dt.float32,
                              kind="ExternalInput").ap()
    wpj_ch_d = nc.dram_tensor("wpj_ch", [WPJ_CH], dt.float32,
                              kind="ExternalInput").ap()
    mask_d = nc.dram_tensor("mask", [128, 4, 2, 512], dt.bfloat16,
                            kind="ExternalInput").ap()
    ln1g_d = nc.dram_tensor("ln1g", [D], dt.float32, kind="ExternalInput").ap()
    ln1b_d = nc.dram_tensor("ln1b", [D], dt.float32, kind="ExternalInput").ap()
    ln2g_d = nc.dram_tensor("ln2g", [D], dt.float32, kind="ExternalInput").ap()
    ln2b_d = nc.dram_tensor("ln2b", [D], dt.float32, kind="ExternalInput").ap()
    bo_d = nc.dram_tensor("bo", [D], dt.float32, kind="ExternalInput").ap()
    bfc_d = nc.dram_tensor("bfc", [FF], dt.float32, kind="ExternalInput").ap()
    bpj_d = nc.dram_tensor("bpj", [D], dt.float32, kind="ExternalInput").ap()
    out_d = nc.dram_tensor("out", [TL, D], dt.float32,
                           kind="ExternalOutput").ap()

    def bcast_row(src_ap, n):
        return bass.AP(tensor=src_ap.tensor, offset=src_ap.offset,
                       ap=[[0, 128], [1, n]])

    with tile.TileContext(nc) as tc:
        import contextlib
        stack = contextlib.ExitStack()
        main = stack.enter_context(tc.tile_pool(name="main", bufs=1))
        dram = stack.enter_context(
            tc.tile_pool(name="dram", bufs=1, space="DRAM"))

        ident = main.tile([128, 128], dt.float32)
        make_identity(nc, ident[:])
        ones_col = main.tile([128, 1], dt.float32)
        nc.vector.memset(ones_col[:], 1.0)
        ones_col_b = main.tile([128, 1], dt.bfloat16)
        nc.vector.tensor_copy(ones_col_b[:], ones_col[:])
        ones_row = main.tile([1, 128], dt.float32)
        nc.vector.memset(ones_row[:], 1.0)
        eps_t = main.tile([128, 1], dt.float32)
        nc.vector.memset(eps_t[:], EPS)
        bo_bc = main.tile([128, D], dt.float32)
        nc.sync.dma_start(out=bo_bc[:], in_=bcast_row(bo_d, D))
        bpj_bc = main.tile([128, D], dt.float32)
        nc.sync.dma_start(out=bpj_bc[:], in_=bcast_row(bpj_d, D))
        ln_bc = {}
        for nm, flag, src in (("g1", apply_g1, ln1g_d),
                              ("b1", apply_b1, ln1b_d),
                              ("g2", apply_g2, ln2g_d),
                              ("b2", apply_b2, ln2b_d)):
            if flag:
                t = main.tile([128, D], dt.float32, name=f"ln_{nm}")
                nc.sync.dma_start(out=t[:], in_=bcast_row(src, D))
                ln_bc[nm] = t
        bfc_pp = main.tile([128, FF // 128], dt.float32)
        nc.sync.dma_start(
            out=bfc_pp[:],
            in_=bass.AP(tensor=bfc_d.tensor, offset=bfc_d.offset,
                        ap=[[1, 128], [128, FF // 128]]))
        mask_pool = tc.tile_pool(name="maskp", bufs=1)
        maskp = mask_pool.__enter__()
        masks = maskp.tile([128, 4, 2, 512], dt.bfloat16)
        nc.sync.dma_start(out=masks[:], in_=mask_d[:])

        # big rotating bf16 slots: hT -> OT reuse, QT -> mT reuse
        hT = main.tile([128, 16, 512], dt.bfloat16, tag="bigA", bufs=2,
                       name="hT")
        QT = main.tile([128, 16, 512], dt.bfloat16, tag="bigA", bufs=2,
                       name="QT")

        h2_d = dram.tile([TL, D], dt.float32)

        def layernorm(x_t, h_t, gk, bk):
            with tc.tile_pool(name="lnp", bufs=2) as lp:
                st = lp.tile([128, 4, 6], dt.float32, tag="st")
                xr = x_t[:].rearrange("p (n f) -> p n f", n=4)
                for sg in range(4):
                    nc.vector.bn_stats(out=st[:, sg, :], in_=xr[:, sg, :])
                mv = lp.tile([128, 2], dt.float32, tag="mv")
                nc.vector.bn_aggr(out=mv[:], in_=st[:])
                std = lp.tile([128, 1], dt.float32, tag="sd")
                nc.scalar.activation(std[:], mv[:, 1:2], AF.Sqrt,
                                     bias=eps_t[:])
                rstd = lp.tile([128, 1], dt.float32, tag="rs")
                nc.vector.reciprocal(rstd[:], std[:])
                nc.vector.tensor_scalar(h_t[:], x_t[:], mv[:, 0:1], rstd[:],
                                        op0=OP.subtract, op1=OP.mult)
                if gk in ln_bc:
                    nc.vector.tensor_mul(h_t[:], h_t[:], ln_bc[gk][:])
                if bk in ln_bc:
                    nc.vector.tensor_add(h_t[:], h_t[:], ln_bc[bk][:])

        # ---------- Phase A: x -> LN1 -> h^T ----------
        with tc.tile_pool(name="xa", bufs=2) as xa, \
             tc.tile_pool(name="ha", bufs=2) as ha, \
             tc.tile_pool(name="trps", bufs=4, space="PSUM") as trps:
            for tb in range(4):
                x_t = xa.tile([128, D], dt.float32, tag="x")
                nc.sync.dma_start(out=x_t[:],
                                  in_=xl_d[tb * 128:(tb + 1) * 128, :])
                h_t = ha.tile([128, D], dt.float32, tag="h")
                layernorm(x_t, h_t, "g1", "b1")
                for dj in range(16):
                    ps = trps.tile([128, 128], dt.float32, tag="tp")
                    nc.tensor.transpose(ps[:], h_t[:, dj * 128:(dj + 1) * 128],
                                        ident[:])
                    nc.vector.tensor_copy(hT[:, dj, tb * 128:(tb + 1) * 128],
                                          ps[:])

        # ---------- Phase B: QKV ----------
        k_bounce = dram.tile([NKV], dt.bfloat16)
        v_bounce = dram.tile([NKV], dt.bfloat16)
        k_gath = dram.tile([NC * NKV], dt.bfloat16, addr_space="Shared")
        v_gath = dram.tile([NC * NKV], dt.bfloat16, addr_space="Shared")

        def project_qk(wT_dram, kind):
            # feature-major output via PE transpose; og(4) x [128,512] loads
            with tc.tile_pool(name=f"pw_{kind}", bufs=4) as wp, \
                 tc.tile_pool(name=f"po_{kind}", bufs=4) as op_, \
                 tc.tile_pool(name=f"pp_{kind}", bufs=1, space="PSUM") as pp, \
                 tc.tile_pool(name=f"pt_{kind}", bufs=4, space="PSUM") as tp2:
                for og in range(4):
                    o_base = og * 512
                    ktacc = []
                    if kind == "k":
                        for k4 in range(4):
                            ka = op_.tile([128, 512], dt.bfloat16, tag="ka",
                                          bufs=8, name=f"ka_{og}_{k4}")
                            ktacc.append(ka)
                    pss = [None] * 4
                    for dj in range(16):
                        raw = wp.tile([128, 512], dt.float32, tag="raw")
                        nc.sync.dma_start(
                            out=raw[:],
                            in_=wT_dram[dj * 128:(dj + 1) * 128,
                                        o_base:o_base + 512])
                        tnh = wp.tile([128, 512], dt.bfloat16, tag="tnh")
                        nc.scalar.activation(tnh[:], raw[:], AF.Tanh)
                        for tb in range(4):
                            if pss[tb] is None:
                                pss[tb] = pp.tile([128, 512], dt.float32,
                                                  tag=f"ps{tb}",
                                                  name=f"ps_{kind}_{tb}")
                            nc.tensor.matmul(
                                pss[tb][:],
                                hT[:, dj, tb * 128:(tb + 1) * 128],
                                tnh[:], start=(dj == 0), stop=(dj == 15))
                    for tb in range(4):
                        tm = op_.tile([128, 512], dt.float32, tag="tm")
                        nc.scalar.activation(tm[:], pss[tb][:], AF.Copy)
                        for k4 in range(4):
                            dj2 = (o_base + k4 * 128) // 128
                            ps2 = tp2.tile([128, 128], dt.float32, tag="t2")
                            nc.tensor.transpose(
                                ps2[:], tm[:, k4 * 128:(k4 + 1) * 128],
                                ident[:])
                            if kind == "q":
                                nc.vector.tensor_copy(
                                    QT[:, dj2, tb * 128:(tb + 1) * 128],
                                    ps2[:])
                            else:
                                nc.vector.tensor_copy(
                                    ktacc[k4][:, tb * 128:(tb + 1) * 128],
                                    ps2[:])
                    if kind == "k":
                        for k4 in range(4):
                            dj2 = (o_base + k4 * 128) // 128
                            nc.sync.dma_start(
                                out=k_bounce[dj2 * 128 * TL:
                                             (dj2 + 1) * 128 * TL]
                                .rearrange("(p t) -> p t", p=128),
                                in_=ktacc[k4][:])

        def project_v(wT_dram):
            # token-major; og2(2) x [128,1024] loads; full-row stores
            with tc.tile_pool(name="pw_v", bufs=4) as wp, \
                 tc.tile_pool(name="po_v", bufs=4) as op_, \
                 tc.tile_pool(name="pp_v", bufs=1, space="PSUM") as pp:
                vacc = [op_.tile([128, D], dt.bfloat16, tag="va", bufs=4,
                                 name=f"va_{t}") for t in range(4)]
                for og2 in range(2):
                    o_base = og2 * 1024
                    pss = [None] * 8
                    for dj in range(16):
                        raw = wp.tile([128, 1024], dt.float32, tag="raw")
                        nc.sync.dma_start(
                            out=raw[:],
                            in_=wT_dram[dj * 128:(dj + 1) * 128,
                                        o_base:o_base + 1024])
                        tnh = wp.tile([128, 1024], dt.bfloat16, tag="tnh")
                        nc.scalar.activation(tnh[:], raw[:], AF.Tanh)
                        for osub in range(2):
                            for tb in range(4):
                                k = osub * 4 + tb
                                if pss[k] is None:
                                    pss[k] = pp.tile([128, 512], dt.float32,
                                                     tag=f"ps{k}",
                                                     name=f"ps_v_{k}")
                                nc.tensor.matmul(
                                    pss[k][:],
                                    hT[:, dj, tb * 128:(tb + 1) * 128],
                                    tnh[:, osub * 512:(osub + 1) * 512],
                                    start=(dj == 0), stop=(dj == 15))
                    for osub in range(2):
                        for tb in range(4):
                            sl = slice(o_base + osub * 512,
                                       o_base + osub * 512 + 512)
                            nc.scalar.activation(vacc[tb][:, sl],
                                                 pss[osub * 4 + tb][:],
                                                 AF.Copy)
                for tb in range(4):
                    nc.sync.dma_start(
                        out=v_bounce[tb * 128 * D:(tb + 1) * 128 * D]
                        .rearrange("(p t) -> p t", p=128),
                        in_=vacc[tb][:])

        project_qk(wkT_d, "k")
        nc.gpsimd.collective_compute(
            "AllGather", OP.bypass, replica_groups=[list(range(NC))],
            ins=[k_bounce[:]], outs=[k_gath[:]])
        project_v(wvT_d)
        nc.gpsimd.collective_compute(
            "AllGather", OP.bypass, replica_groups=[list(range(NC))],
            ins=[v_bounce[:]], outs=[v_gath[:]])
        project_qk(wqT_d, "q")

        # ---------- MLP weight tanh (own 1/8th) + AllGathers ----------
        wfc_bounce = dram.tile([WFC_CH], dt.bfloat16)
        wpj_bounce = dram.tile([WPJ_CH], dt.bfloat16)
        wfc_gath = dram.tile([NC * WFC_CH], dt.bfloat16, addr_space="Shared")
        wpj_gath = dram.tile([NC * WPJ_CH], dt.bfloat16, addr_space="Shared")
        with tc.tile_pool(name="wprep", bufs=3) as wprep:
            for src, dst, odt, n_t, otag in (
                    (wfc_ch_d, wfc_bounce, dt.bfloat16,
                     WFC_CH // (128 * 2048), "f"),
                    (wpj_ch_d, wpj_bounce, dt.bfloat16,
                     WPJ_CH // (128 * 2048), "p")):
                for i in range(n_t):
                    raw = wprep.tile([128, 2048], dt.float32, tag="wraw")
                    nc.sync.dma_start(
                        out=raw[:],
                        in_=src[i * 128 * 2048:(i + 1) * 128 * 2048]
                        .rearrange("(p f) -> p f", p=128))
                    tnh = wprep.tile([128, 2048], odt, tag=f"wtnh{otag}")
                    nc.scalar.activation(tnh[:], raw[:], AF.Tanh)
                    nc.sync.dma_start(
                        out=dst[i * 128 * 2048:(i + 1) * 128 * 2048]
                        .rearrange("(p f) -> p f", p=128), in_=tnh[:])
        nc.gpsimd.collective_compute(
            "AllGather", OP.bypass, replica_groups=[list(range(NC))],
            ins=[wfc_bounce[:]], outs=[wfc_gath[:]])
        nc.gpsimd.collective_compute(
            "AllGather", OP.bypass, replica_groups=[list(range(NC))],
            ins=[wpj_bounce[:]], outs=[wpj_gath[:]])
        wfcT_v = wfc_gath[:].rearrange("(d f) -> d f", d=D)    # [D, FF]
        wpjT_v = wpj_gath[:].rearrange("(f o) -> f o", f=FF)   # [FF, D]


        # ---------- Phase C: attention (bf16) ----------
        OT = main.tile([128, 16, 512], dt.bfloat16, tag="bigA", bufs=2,
                       name="OT")
        with tc.tile_pool(name="kvh", bufs=3) as kvh, \
             tc.tile_pool(name="att", bufs=4) as att, \
             tc.tile_pool(name="attsm", bufs=6) as attsm, \
             tc.tile_pool(name="stps", bufs=3, space="PSUM") as stps, \
             tc.tile_pool(name="otps", bufs=2, space="PSUM") as otps, \
             tc.tile_pool(name="dnps", bufs=2, space="PSUM") as dnps, \
             tc.tile_pool(name="bcps", bufs=1, space="PSUM") as bcps:
            for hg in range(4):            # head groups of 4
                kt_g, v_g = [], []
                for j in range(NC):
                    kt = kvh.tile([128, 4, 512], dt.bfloat16, tag="kth",
                                  bufs=12, name=f"kt_{hg}_{j}")
                    nc.sync.dma_start(
                        out=kt[:],
                        in_=bass.AP(tensor=k_gath.tensor,
                                    offset=k_gath.offset + j * NKV
                                    + hg * 4 * 128 * TL,
                                    ap=[[TL, 128], [128 * TL, 4], [1, TL]]))
                    kt_g.append(kt)
                    vt = kvh.tile([128, 4, 512], dt.bfloat16, tag="vth",
                                  bufs=12, name=f"vt_{hg}_{j}")
                    nc.sync.dma_start(
                        out=vt[:],
                        in_=bass.AP(tensor=v_gath.tensor,
                                    offset=v_gath.offset + j * NKV
                                    + hg * 4 * 128,
                                    ap=[[D, 128], [128 * D, 4], [1, 512]]))
                    v_g.append(vt)
                for hh in range(4):
                    h = hg * 4 + hh
                    for b in range(2):
                        ot_ps = otps.tile([128, 256], dt.float32, tag="ot")
                        dn_ps = dnps.tile([1, 256], dt.float32, tag="dn")
                        n_acc = 0
                        for tb in range(2):
                            for jp in range(4):
                                st = stps.tile([128, 512], dt.float32,
                                               tag="st")
                                for half in range(2):
                                    j = 2 * jp + half
                                    nc.tensor.matmul(
                                        st[:, half * 256:(half + 1) * 256],
                                        kt_g[j][:, hh,
                                                b * 256 + tb * 128:
                                                b * 256 + tb * 128 + 128],
                                        QT[:, h, b * 256:(b + 1) * 256],
                                        start=True, stop=True)
                                pt_pre = attsm.tile([128, 512], dt.bfloat16,
                                                    tag="ptp")
                                nc.vector.tensor_add(pt_pre[:], st[:],
                                                     masks[:, jp, tb, :])
                                pt = attsm.tile([128, 512], dt.bfloat16,
                                                tag="pt")
                                nc.scalar.activation(pt[:], pt_pre[:], AF.Exp)
                                for half in range(2):
                                    j = 2 * jp + half
                                    last = (tb == 1 and jp == 3 and half == 1)
                                    nc.tensor.matmul(
                                        ot_ps[:],
                                        v_g[j][:, 2 * b + tb,
                                               hh * 128:(hh + 1) * 128],
                                        pt[:, half * 256:(half + 1) * 256],
                                        start=(n_acc == 0), stop=last,
                                        skip_group_check=True)
                                    nc.tensor.matmul(
                                        dn_ps[:], ones_col_b[:],
                                        pt[:, half * 256:(half + 1) * 256],
                                        start=(n_acc == 0), stop=last,
                                        skip_group_check=True)
                                    n_acc += 1
                        dn_sb = att.tile([1, 256], dt.float32, tag="dns")
                        nc.vector.reciprocal(dn_sb[:], dn_ps[:])
                        bc_ps = bcps.tile([128, 256], dt.float32, tag="bc")
                        nc.tensor.matmul(bc_ps[:], ones_row[:], dn_sb[:],
                                         start=True, stop=True)
                        bc_sb = att.tile([128, 256], dt.float32, tag="bcs")
                        nc.vector.tensor_copy(bc_sb[:], bc_ps[:])
                        nc.vector.tensor_mul(OT[:, h, b * 256:(b + 1) * 256],
                                             ot_ps[:], bc_sb[:])

        mask_pool.__exit__(None, None, None)

        # ---------- Phase D: out-proj + residual + LN2 -> m^T ----------
        mT = main.tile([128, 16, 512], dt.bfloat16, tag="bigA", bufs=2,
                       name="mT")
        h2_pool = tc.tile_pool(name="h2a", bufs=4)
        h2a = h2_pool.__enter__()
        h2acc = [h2a.tile([128, D], dt.float32, tag="h2", bufs=4,
                          name=f"h2_{t}") for t in range(4)]
        with tc.tile_pool(name="wo", bufs=3) as wop, \
             tc.tile_pool(name="zps", bufs=1, space="PSUM") as zps:
            for og2 in range(2):
                o_base = og2 * 1024
                pss = [None] * 8
                for dj in range(16):
                    raw = wop.tile([128, 1024], dt.float32, tag="raw")
                    nc.sync.dma_start(
                        out=raw[:], in_=woT_d[dj * 128:(dj + 1) * 128,
                                              o_base:o_base + 1024])
                    tnh = wop.tile([128, 1024], dt.bfloat16, tag="tnh")
                    nc.scalar.activation(tnh[:], raw[:], AF.Tanh)
                    for osub in range(2):
                        for tb in range(4):
                            k = osub * 4 + tb
                            if pss[k] is None:
                                pss[k] = zps.tile([128, 512], dt.float32,
                                                  tag=f"z{k}", name=f"z_{k}")
                            nc.tensor.matmul(
                                pss[k][:],
                                OT[:, dj, tb * 128:(tb + 1) * 128],
                                tnh[:, osub * 512:(osub + 1) * 512],
                                start=(dj == 0), stop=(dj == 15))
                for osub in range(2):
                    for tb in range(4):
                        sl = slice(o_base + osub * 512,
                                   o_base + osub * 512 + 512)
                        nc.vector.tensor_add(h2acc[tb][:, sl],
                                             pss[osub * 4 + tb][:],
                                             bo_bc[:, sl])
        with tc.tile_pool(name="xd", bufs=2) as xd, \
             tc.tile_pool(name="md", bufs=1) as md, \
             tc.tile_pool(name="trps2", bufs=4, space="PSUM") as trps2:
            for tb in range(4):
                for xh in range(2):
                    x_t = xd.tile([128, 1024], dt.float32, tag="x2")
                    nc.sync.dma_start(
                        out=x_t[:],
                        in_=xl_d[tb * 128:(tb + 1) * 128,
                                 xh * 1024:(xh + 1) * 1024])
                    nc.vector.tensor_add(
                        h2acc[tb][:, xh * 1024:(xh + 1) * 1024],
                        h2acc[tb][:, xh * 1024:(xh + 1) * 1024], x_t[:])
                nc.sync.dma_start(out=h2_d[tb * 128:(tb + 1) * 128, :],
                                  in_=h2acc[tb][:])
                m_t = md.tile([128, D], dt.float32, tag="m")
                layernorm(h2acc[tb], m_t, "g2", "b2")
                for dj in range(16):
                    ps = trps2.tile([128, 128], dt.float32, tag="tp2")
                    nc.tensor.transpose(ps[:], m_t[:, dj * 128:(dj + 1) * 128],
                                        ident[:])
                    nc.vector.tensor_copy(mT[:, dj, tb * 128:(tb + 1) * 128],
                                          ps[:])

        h2_pool.__exit__(None, None, None)

        # ---------- Phase E: MLP ----------
        gt_pool = tc.tile_pool(name="gtpl", bufs=1)
        gtpl = gt_pool.__enter__()
        GT1 = gtpl.tile([128, 32, 512], dt.bfloat16, name="GT1")
        GT2 = gtpl.tile([128, 32, 512], dt.bfloat16, name="GT2")

        def gt_slice(fti, c0, c1):
            if fti < 32:
                return GT1[:, fti, c0:c1]
            return GT2[:, fti - 32, c0:c1]

        if True:
            with tc.tile_pool(name="wfc", bufs=8) as wfcp, \
                 tc.tile_pool(name="ups", bufs=1, space="PSUM") as ups:
                for FG in range(8):            # 1024 f-cols per group
                    pss = [None] * 8
                    for dj in range(16):
                        wt = wfcp.tile([128, 1024], dt.bfloat16, tag="wfct")
                        nc.sync.dma_start(
                            out=wt[:],
                            in_=wfcT_v[dj * 128:(dj + 1) * 128,
                                       FG * 1024:(FG + 1) * 1024])
                        for fsub in range(8):
                            if pss[fsub] is None:
                                pss[fsub] = ups.tile([128, 512], dt.float32,
                                                     tag=f"u{fsub}",
                                                     name=f"u_{fsub}")
                            nc.tensor.matmul(
                                pss[fsub][:],
                                wt[:, fsub * 128:(fsub + 1) * 128],
                                mT[:, dj, :],
                                start=(dj == 0), stop=(dj == 15))
                    for fsub in range(8):
                        fti = FG * 8 + fsub
                        nc.scalar.activation(gt_slice(fti, 0, 512),
                                             pss[fsub][:],
                                             AF.Gelu_apprx_tanh,
                                             bias=bfc_pp[:, fti:fti + 1])
            with tc.tile_pool(name="wpj", bufs=5) as wpjp, \
                 tc.tile_pool(name="yps", bufs=1, space="PSUM") as yps, \
                 tc.tile_pool(name="outp", bufs=6) as outp:
                for tg in range(2):            # tt groups of 2
                    pss = {}
                    h2s_g = {}
                    for ft in range(64):
                        wt = wpjp.tile([128, D], dt.bfloat16, tag="wpjt")
                        nc.sync.dma_start(
                            out=wt[:], in_=wpjT_v[ft * 128:(ft + 1) * 128, :])
                        for ob in range(4):
                            for ti in range(2):
                                tt = tg * 2 + ti
                                key = (ob, ti)
                                if key not in pss:
                                    pss[key] = yps.tile(
                                        [128, 512], dt.float32,
                                        tag=f"y{ob}{ti}", name=f"y_{ob}_{ti}")
                                nc.tensor.matmul(
                                    pss[key][:],
                                    gt_slice(ft, tt * 128, (tt + 1) * 128),
                                    wt[:, ob * 512:(ob + 1) * 512],
                                    start=(ft == 0), stop=(ft == 63))
                    for ti in range(2):
                        tt = tg * 2 + ti
                        h2s = outp.tile([128, D], dt.float32, tag="h2s",
                                        bufs=2, name=f"h2s_{tt}")
                        nc.sync.dma_start(
                            out=h2s[:], in_=h2_d[tt * 128:(tt + 1) * 128, :])
                        h2s_g[ti] = h2s
                    for ob in range(4):
                        for ti in range(2):
                            tt = tg * 2 + ti
                            sl = slice(ob * 512, ob * 512 + 512)
                            o_t = outp.tile([128, 512], dt.float32, tag="o")
                            nc.vector.tensor_add(o_t[:], pss[(ob, ti)][:],
                                                 bpj_bc[:, sl])
                            nc.vector.tensor_add(o_t[:], o_t[:],
                                                 h2s_g[ti][:, sl])
                            nc.sync.dma_start(
                                out=out_d[tt * 128:(tt + 1) * 128, sl],
                                in_=o_t[:])
        gt_pool.__exit__(None, None, None)
        stack.close()

    nc.compile()
    return nc


def _host_prep(inputs):
    f32 = lambda k: np.ascontiguousarray(np.asarray(inputs[k], np.float32))
    x = f32("hidden_states")
    wqT = np.ascontiguousarray(f32("wq").T)
    wkT = np.ascontiguousarray(f32("wk").T)
    wvT = np.ascontiguousarray(f32("wv").T)
    woT = np.ascontiguousarray(f32("wo").T)
    wfcT = np.ascontiguousarray(f32("w_fc").T).ravel()
    wpjT = np.ascontiguousarray(f32("w_proj").T).ravel()
    kp = np.arange(128)
    q_f = np.arange(256)
    in_maps = []
    for c in range(NC):
        mask = np.empty((128, 4, 2, 512), np.float32)
        for jp in range(4):
            for tb in range(2):
                for half in range(2):
                    j = 2 * jp + half
                    ktok = 8 * (128 * tb + kp)[:, None] + j
                    qtok = 8 * q_f[None, :] + c
                    mask[:, jp, tb, half * 256:(half + 1) * 256] = np.where(
                        ktok <= qtok, 0.0, -1e9)
        in_maps.append({
            "xl": np.concatenate([x[0, c::NC, :], x[1, c::NC, :]], 0),
            "wqT": wqT, "wkT": wkT, "wvT": wvT, "woT": woT,
            "wfc_ch": wfcT[c * WFC_CH:(c + 1) * WFC_CH],
            "wpj_ch": wpjT[c * WPJ_CH:(c + 1) * WPJ_CH],
            "mask": mask.astype(ml_dtypes.bfloat16),
            "ln1g": f32("ln1_g"), "ln1b": f32("ln1_b"),
            "ln2g": f32("ln2_g"), "ln2b": f32("ln2_b"),
            "bo": f32("bo"), "bfc": f32("b_fc"), "bpj": f32("b_proj"),
        })
    return in_maps


def kernel(**inputs) -> np.ndarray:
    in_maps = _host_prep(inputs)
    key = (not bool(np.all(np.asarray(inputs["ln1_g"]) == 1.0)),
           not bool(np.all(np.asarray(inputs["ln1_b"]) == 0.0)),
           not bool(np.all(np.asarray(inputs["ln2_g"]) == 1.0)),
           not bool(np.all(np.asarray(inputs["ln2_b"]) == 0.0)))
    if key not in _CACHE:
        _CACHE[key] = _build(*key)
    nc = _CACHE[key]
    res = run_bass_kernel_spmd(nc, in_maps, core_ids=list(range(NC)))
    if res.exec_time_ns is not None:
        print(f"HW exec time: {res.exec_time_ns} ns")
    out = np.zeros((B, S, D), np.float32)
    for c in range(NC):
        o = res.results[c]["out"]
        out[0, c::NC] = o[:RPC]
        out[1, c::NC] = o[RPC:]
    return out



# revision 8
# speedup vs baseline: 1.1689x; 1.1689x over previous
"""BinaryGPTNeoBlock on 8 trn2 NeuronCores.

Sequence-parallel over 8 cores: core c owns rows {c, c+8, ...} of both
batch elements (256 per batch, 512 total). Collectives (one stream, in
program order): AllGather K (bf16), V (fp8), then tanh'd+scaled fp8
out-proj/fc/proj weights -- all triggered early so they hide under the
QKV projections and attention. Out-proj and both MLP matmuls run fp8
DoubleRow (weights scaled x1024 into e4m3 range, descaled on PSUM
read); q/k/scores stay bf16 for softmax fidelity. Attention exp reads
PSUM directly and causal masking is a multiplicative bf16 {0,1} mask
applied after exp.

Self-contained: hardcodes shapes; host only shards/transposes/builds masks.
"""

import numpy as np
import ml_dtypes

import concourse.bass as bass
import concourse.tile as tile
from concourse import bacc, mybir
from concourse.bass_utils import run_bass_kernel_spmd
from concourse.masks import make_identity

B, S, D = 2, 2048, 2048
H = 16
HD = 128
FF = 4 * D
EPS = 1e-5
NC = 8
RPC = S // NC          # 256 rows per core per batch
TL = 2 * RPC           # 512 local rows
NKV = TL * D           # elems of K^T (== of V) per core
WO_CH = D * D // NC
WFC_CH = D * FF // NC
WPJ_CH = FF * D // NC
WSCALE = 1024.0        # fp8 weight scale (w in +-0.0221 -> +-22.6)

dt = mybir.dt
AF = mybir.ActivationFunctionType
OP = mybir.AluOpType
PM = mybir.MatmulPerfMode

_CACHE = {}


def _build(apply_g1, apply_b1, apply_g2, apply_b2):
    nc = bacc.Bacc("TRN2", target_bir_lowering=False, debug=False,
                   num_devices=NC)

    xl_d = nc.dram_tensor("xl", [TL, D], dt.float32, kind="ExternalInput").ap()
    wqT_d = nc.dram_tensor("wqT", [D, D], dt.float32, kind="ExternalInput").ap()
    wkT_d = nc.dram_tensor("wkT", [D, D], dt.float32, kind="ExternalInput").ap()
    wvT_d = nc.dram_tensor("wvT", [D, D], dt.float32, kind="ExternalInput").ap()
    wo_ch_d = nc.dram_tensor("wo_ch", [WO_CH], dt.float32,
                             kind="ExternalInput").ap()
    wfc_ch_d = nc.dram_tensor("wfc_ch", [WFC_CH], dt.float32,
                              kind="ExternalInput").ap()
    wpj_ch_d = nc.dram_tensor("wpj_ch", [WPJ_CH], dt.float32,
                              kind="ExternalInput").ap()
    mask_d = nc.dram_tensor("mask", [128, 4, 2, 512], dt.bfloat16,
                            kind="ExternalInput").ap()
    ln1g_d = nc.dram_tensor("ln1g", [D], dt.float32, kind="ExternalInput").ap()
    ln1b_d = nc.dram_tensor("ln1b", [D], dt.float32, kind="ExternalInput").ap()
    ln2g_d = nc.dram_tensor("ln2g", [D], dt.float32, kind="ExternalInput").ap()
    ln2b_d = nc.dram_tensor("ln2b", [D], dt.float32, kind="ExternalInput").ap()
    bo_d = nc.dram_tensor("bo", [D], dt.float32, kind="ExternalInput").ap()
    bfc_d = nc.dram_tensor("bfc", [FF], dt.float32, kind="ExternalInput").ap()
    bpj_d = nc.dram_tensor("bpj", [D], dt.float32, kind="ExternalInput").ap()
    out_d = nc.dram_tensor("out", [TL, D], dt.float32,
                           kind="ExternalOutput").ap()

    def bcast_row(src_ap, n):
        return bass.AP(tensor=src_ap.tensor, offset=src_ap.offset,
                       ap=[[0, 128], [1, n]])

    with tile.TileContext(nc) as tc:
        import contextlib
        stack = contextlib.ExitStack()
        main = stack.enter_context(tc.tile_pool(name="main", bufs=1))
        dram = stack.enter_context(
            tc.tile_pool(name="dram", bufs=1, space="DRAM"))

        ident = main.tile([128, 128], dt.float32)
        make_identity(nc, ident[:])
        ones_col = main.tile([128, 1], dt.float32)
        nc.vector.memset(ones_col[:], 1.0)
        ones_col_b = main.tile([128, 1], dt.bfloat16)
        nc.vector.tensor_copy(ones_col_b[:], ones_col[:])
        ones_row = main.tile([1, 128], dt.float32)
        nc.vector.memset(ones_row[:], 1.0)
        eps_t = main.tile([128, 1], dt.float32)
        nc.vector.memset(eps_t[:], EPS)
        bo_bc = main.tile([128, D], dt.float32)
        nc.sync.dma_start(out=bo_bc[:], in_=bcast_row(bo_d, D))
        bpj_bc = main.tile([128, D], dt.float32)
        nc.sync.dma_start(out=bpj_bc[:], in_=bcast_row(bpj_d, D))
        ln_bc = {}
        for nm, flag, src in (("g1", apply_g1, ln1g_d),
                              ("b1", apply_b1, ln1b_d),
                              ("g2", apply_g2, ln2g_d),
                              ("b2", apply_b2, ln2b_d)):
            if flag:
                t = main.tile([128, D], dt.float32, name=f"ln_{nm}")
                nc.sync.dma_start(out=t[:], in_=bcast_row(src, D))
                ln_bc[nm] = t
        bfc_pp = main.tile([128, FF // 128], dt.float32)
        nc.sync.dma_start(
            out=bfc_pp[:],
            in_=bass.AP(tensor=bfc_d.tensor, offset=bfc_d.offset,
                        ap=[[1, 128], [128, FF // 128]]))
        mask_pool = tc.tile_pool(name="maskp", bufs=1)
        maskp = mask_pool.__enter__()
        masks = maskp.tile([128, 4, 2, 512], dt.bfloat16)
        nc.sync.dma_start(out=masks[:], in_=mask_d[:])

        # Long-lived K/V staging pool (created before phase-B pools so its
        # SBUF doesn't alias them; sized for 2 head-groups of prefetch).
        kvh_pool = tc.tile_pool(name="kvh", bufs=1)
        kvh = kvh_pool.__enter__()

        # big rotating slots: hT -> OT reuse, QT -> mT reuse
        hT = main.tile([128, 16, 512], dt.bfloat16, tag="bigA", bufs=2,
                       name="hT")
        QT = main.tile([128, 16, 512], dt.bfloat16, tag="bigA", bufs=2,
                       name="QT")

        h2_d = dram.tile([TL, D], dt.float32)

        def layernorm(x_t, h_t, gk, bk):
            with tc.tile_pool(name="lnp", bufs=2) as lp:
                st = lp.tile([128, 4, 6], dt.float32, tag="st")
                xr = x_t[:].rearrange("p (n f) -> p n f", n=4)
                for sg in range(4):
                    nc.vector.bn_stats(out=st[:, sg, :], in_=xr[:, sg, :])
                mv = lp.tile([128, 2], dt.float32, tag="mv")
                nc.vector.bn_aggr(out=mv[:], in_=st[:])
                std = lp.tile([128, 1], dt.float32, tag="sd")
                nc.scalar.activation(std[:], mv[:, 1:2], AF.Sqrt,
                                     bias=eps_t[:])
                rstd = lp.tile([128, 1], dt.float32, tag="rs")
                nc.vector.reciprocal(rstd[:], std[:])
                nc.vector.tensor_scalar(h_t[:], x_t[:], mv[:, 0:1], rstd[:],
                                        op0=OP.subtract, op1=OP.mult)
                if gk in ln_bc:
                    nc.vector.tensor_mul(h_t[:], h_t[:], ln_bc[gk][:])
                if bk in ln_bc:
                    nc.vector.tensor_add(h_t[:], h_t[:], ln_bc[bk][:])

        # ---------- Phase A: x -> LN1 -> h^T ----------
        with tc.tile_pool(name="xa", bufs=2) as xa, \
             tc.tile_pool(name="ha", bufs=2) as ha, \
             tc.tile_pool(name="trps", bufs=4, space="PSUM") as trps:
            for tb in range(4):
                x_t = xa.tile([128, D], dt.float32, tag="x")
                nc.sync.dma_start(out=x_t[:],
                                  in_=xl_d[tb * 128:(tb + 1) * 128, :])
                h_t = ha.tile([128, D], dt.float32, tag="h")
                layernorm(x_t, h_t, "g1", "b1")
                for dj in range(16):
                    ps = trps.tile([128, 128], dt.float32, tag="tp")
                    nc.tensor.transpose(ps[:], h_t[:, dj * 128:(dj + 1) * 128],
                                        ident[:])
                    nc.vector.tensor_copy(hT[:, dj, tb * 128:(tb + 1) * 128],
                                          ps[:])

        # ---------- wprep: tanh+scale fp8 chunks of wo/wfc/wpj ----------
        # (issued early; runs in DMA/ACT gaps of phases A-B, so the three
        # weight gathers can trigger right after the V gather)
        wo_bounce = dram.tile([WO_CH], dt.float8e4)
        wfc_bounce = dram.tile([WFC_CH], dt.float8e4)
        wpj_bounce = dram.tile([WPJ_CH], dt.float8e4)
        with tc.tile_pool(name="wprep", bufs=2) as wprep:
            for src, dst, n_t, otag in (
                    (wo_ch_d, wo_bounce, WO_CH // (128 * 2048), "o"),
                    (wfc_ch_d, wfc_bounce, WFC_CH // (128 * 2048), "f"),
                    (wpj_ch_d, wpj_bounce, WPJ_CH // (128 * 2048), "p")):
                for i in range(n_t):
                    raw = wprep.tile([128, 2048], dt.float32, tag="wraw")
                    nc.sync.dma_start(
                        out=raw[:],
                        in_=src[i * 128 * 2048:(i + 1) * 128 * 2048]
                        .rearrange("(p f) -> p f", p=128))
                    tnh = wprep.tile([128, 2048], dt.bfloat16, tag="wtnh")
                    nc.scalar.activation(tnh[:], raw[:], AF.Tanh)
                    sc8 = wprep.tile([128, 2048], dt.float8e4, tag="wsc")
                    nc.vector.tensor_scalar_mul(sc8[:], tnh[:], WSCALE)
                    nc.sync.dma_start(
                        out=dst[i * 128 * 2048:(i + 1) * 128 * 2048]
                        .rearrange("(p f) -> p f", p=128), in_=sc8[:])

        # ---------- Phase B: QKV ----------
        k_bounce = dram.tile([NKV], dt.bfloat16)
        v_bounce = dram.tile([NKV], dt.float8e4)
        k_gath = dram.tile([NC * NKV], dt.bfloat16, addr_space="Shared")
        v_gath = dram.tile([NC * NKV], dt.float8e4, addr_space="Shared")
        wo_gath = dram.tile([NC * WO_CH], dt.float8e4, addr_space="Shared")
        wfc_gath = dram.tile([NC * WFC_CH], dt.float8e4, addr_space="Shared")
        wpj_gath = dram.tile([NC * WPJ_CH], dt.float8e4, addr_space="Shared")

        def project_qk(wT_dram, kind):
            # feature-major output via PE transpose; og(4) x [128,512] loads
            with tc.tile_pool(name=f"pw_{kind}", bufs=4) as wp, \
                 tc.tile_pool(name=f"po_{kind}", bufs=4) as op_, \
                 tc.tile_pool(name=f"pp_{kind}", bufs=1, space="PSUM") as pp, \
                 tc.tile_pool(name=f"pt_{kind}", bufs=4, space="PSUM") as tp2:
                for og in range(4):
                    o_base = og * 512
                    ktacc = []
                    if kind == "k":
                        for k4 in range(4):
                            ka = op_.tile([128, 512], dt.bfloat16, tag="ka",
                                          bufs=8, name=f"ka_{og}_{k4}")
                            ktacc.append(ka)
                    pss = [None] * 4
                    for dj in range(16):
                        raw = wp.tile([128, 512], dt.float32, tag="raw")
                        nc.sync.dma_start(
                            out=raw[:],
                            in_=wT_dram[dj * 128:(dj + 1) * 128,
                                        o_base:o_base + 512])
                        tnh = wp.tile([128, 512], dt.bfloat16, tag="tnh")
                        nc.scalar.activation(tnh[:], raw[:], AF.Tanh)
                        for tb in range(4):
                            if pss[tb] is None:
                                pss[tb] = pp.tile([128, 512], dt.float32,
                                                  tag=f"ps{tb}",
                                                  name=f"ps_{kind}_{tb}")
                            nc.tensor.matmul(
                                pss[tb][:],
                                hT[:, dj, tb * 128:(tb + 1) * 128],
                                tnh[:], start=(dj == 0), stop=(dj == 15))
                    for tb in range(4):
                        tm = op_.tile([128, 512], dt.float32, tag="tm")
                        nc.scalar.activation(tm[:], pss[tb][:], AF.Copy)
                        for k4 in range(4):
                            dj2 = (o_base + k4 * 128) // 128
                            ps2 = tp2.tile([128, 128], dt.float32, tag="t2")
                            nc.tensor.transpose(
                                ps2[:], tm[:, k4 * 128:(k4 + 1) * 128],
                                ident[:])
                            if kind == "q":
                                nc.vector.tensor_copy(
                                    QT[:, dj2, tb * 128:(tb + 1) * 128],
                                    ps2[:])
                            else:
                                nc.vector.tensor_copy(
                                    ktacc[k4][:, tb * 128:(tb + 1) * 128],
                                    ps2[:])
                    if kind == "k":
                        for k4 in range(4):
                            dj2 = (o_base + k4 * 128) // 128
                            nc.sync.dma_start(
                                out=k_bounce[dj2 * 128 * TL:
                                             (dj2 + 1) * 128 * TL]
                                .rearrange("(p t) -> p t", p=128),
                                in_=ktacc[k4][:])

        def project_v(wT_dram):
            # token-major; og2(2) x [128,1024] loads; full-row fp8 stores
            with tc.tile_pool(name="pw_v", bufs=4) as wp, \
                 tc.tile_pool(name="po_v", bufs=4) as op_, \
                 tc.tile_pool(name="pp_v", bufs=1, space="PSUM") as pp:
                vacc = [op_.tile([128, D], dt.float8e4, tag="va", bufs=4,
                                 name=f"va_{t}") for t in range(4)]
                for og2 in range(2):
                    o_base = og2 * 1024
                    pss = [None] * 8
                    for dj in range(16):
                        raw = wp.tile([128, 1024], dt.float32, tag="raw")
                        nc.sync.dma_start(
                            out=raw[:],
                            in_=wT_dram[dj * 128:(dj + 1) * 128,
                                        o_base:o_base + 1024])
                        tnh = wp.tile([128, 1024], dt.bfloat16, tag="tnh")
                        nc.scalar.activation(tnh[:], raw[:], AF.Tanh)
                        for osub in range(2):
                            for tb in range(4):
                                k = osub * 4 + tb
                                if pss[k] is None:
                                    pss[k] = pp.tile([128, 512], dt.float32,
                                                     tag=f"ps{k}",
                                                     name=f"ps_v_{k}")
                                nc.tensor.matmul(
                                    pss[k][:],
                                    hT[:, dj, tb * 128:(tb + 1) * 128],
                                    tnh[:, osub * 512:(osub + 1) * 512],
                                    start=(dj == 0), stop=(dj == 15))
                    for osub in range(2):
                        for tb in range(4):
                            sl = slice(o_base + osub * 512,
                                       o_base + osub * 512 + 512)
                            nc.scalar.activation(vacc[tb][:, sl],
                                                 pss[osub * 4 + tb][:],
                                                 AF.Copy)
                for tb in range(4):
                    nc.sync.dma_start(
                        out=v_bounce[tb * 128 * D:(tb + 1) * 128 * D]
                        .rearrange("(p t) -> p t", p=128),
                        in_=vacc[tb][:])

        project_qk(wkT_d, "k")
        nc.gpsimd.collective_compute(
            "AllGather", OP.bypass, replica_groups=[list(range(NC))],
            ins=[k_bounce[:]], outs=[k_gath[:]])
        project_v(wvT_d)
        nc.gpsimd.collective_compute(
            "AllGather", OP.bypass, replica_groups=[list(range(NC))],
            ins=[v_bounce[:]], outs=[v_gath[:]])
        nc.gpsimd.collective_compute(
            "AllGather", OP.bypass, replica_groups=[list(range(NC))],
            ins=[wo_bounce[:]], outs=[wo_gath[:]])
        nc.gpsimd.collective_compute(
            "AllGather", OP.bypass, replica_groups=[list(range(NC))],
            ins=[wfc_bounce[:]], outs=[wfc_gath[:]])
        nc.gpsimd.collective_compute(
            "AllGather", OP.bypass, replica_groups=[list(range(NC))],
            ins=[wpj_bounce[:]], outs=[wpj_gath[:]])
        project_qk(wqT_d, "q")

        # ---------- Phase C: attention ----------
        OT = main.tile([128, 16, 512], dt.float8e4, tag="bigB", bufs=2,
                       name="OT")
        with tc.tile_pool(name="att", bufs=4) as att, \
             tc.tile_pool(name="attsm", bufs=6) as attsm, \
             tc.tile_pool(name="stps", bufs=3, space="PSUM") as stps, \
             tc.tile_pool(name="otps", bufs=2, space="PSUM") as otps, \
             tc.tile_pool(name="dnps", bufs=2, space="PSUM") as dnps, \
             tc.tile_pool(name="bcps", bufs=1, space="PSUM") as bcps:
            for hg in range(4):            # head groups of 4
                kt_g, v_g = [], []
                for j in range(NC):
                    kt = kvh.tile([128, 4, 512], dt.bfloat16, tag="kth",
                                  bufs=12, name=f"kt_{hg}_{j}")
                    nc.sync.dma_start(
                        out=kt[:],
                        in_=bass.AP(tensor=k_gath.tensor,
                                    offset=k_gath.offset + j * NKV
                                    + hg * 4 * 128 * TL,
                                    ap=[[TL, 128], [128 * TL, 4], [1, TL]]))
                    kt_g.append(kt)
                    vt = kvh.tile([128, 4, 512], dt.float8e4, tag="vth",
                                  bufs=12, name=f"vt_{hg}_{j}")
                    nc.sync.dma_start(
                        out=vt[:],
                        in_=bass.AP(tensor=v_gath.tensor,
                                    offset=v_gath.offset + j * NKV
                                    + hg * 4 * 128,
                                    ap=[[D, 128], [128 * D, 4], [1, 512]]))
                    v_g.append(vt)
                for hh in range(4):
                    h = hg * 4 + hh
                    for b in range(2):
                        ot_ps = otps.tile([128, 256], dt.float32, tag="ot")
                        dn_ps = dnps.tile([1, 256], dt.float32, tag="dn")
                        n_acc = 0
                        for tb in range(2):
                            for jp in range(4):
                                st = stps.tile([128, 512], dt.float32,
                                               tag="st")
                                for half in range(2):
                                    j = 2 * jp + half
                                    nc.tensor.matmul(
                                        st[:, half * 256:(half + 1) * 256],
                                        kt_g[j][:, hh,
                                                b * 256 + tb * 128:
                                                b * 256 + tb * 128 + 128],
                                        QT[:, h, b * 256:(b + 1) * 256],
                                        start=True, stop=True)
                                pt_e = attsm.tile([128, 512], dt.bfloat16,
                                                  tag="pte")
                                nc.scalar.activation(pt_e[:], st[:], AF.Exp)
                                pt = attsm.tile([128, 512], dt.bfloat16,
                                                tag="pt")
                                nc.vector.tensor_mul(pt[:], pt_e[:],
                                                     masks[:, jp, tb, :])
                                for half in range(2):
                                    j = 2 * jp + half
                                    last = (tb == 1 and jp == 3 and half == 1)
                                    nc.tensor.matmul(
                                        ot_ps[:],
                                        v_g[j][:, 2 * b + tb,
                                               hh * 128:(hh + 1) * 128],
                                        pt[:, half * 256:(half + 1) * 256],
                                        start=(n_acc == 0), stop=last,
                                        skip_group_check=True)
                                    nc.tensor.matmul(
                                        dn_ps[:], ones_col_b[:],
                                        pt[:, half * 256:(half + 1) * 256],
                                        start=(n_acc == 0), stop=last,
                                        skip_group_check=True)
                                    n_acc += 1
                        dn_sb = att.tile([1, 256], dt.float32, tag="dns")
                        nc.vector.reciprocal(dn_sb[:], dn_ps[:])
                        bc_ps = bcps.tile([128, 256], dt.float32, tag="bc")
                        nc.tensor.matmul(bc_ps[:], ones_row[:], dn_sb[:],
                                         start=True, stop=True)
                        bc_sb = att.tile([128, 256], dt.float32, tag="bcs")
                        nc.vector.tensor_copy(bc_sb[:], bc_ps[:])
                        nc.vector.tensor_mul(OT[:, h, b * 256:(b + 1) * 256],
                                             ot_ps[:], bc_sb[:])

        kvh_pool.__exit__(None, None, None)
        mask_pool.__exit__(None, None, None)

        wo_v = wo_gath[:].rearrange("(d o) -> d o", d=D)       # [D, D]
        wfcT_v = wfc_gath[:].rearrange("(d f) -> d f", d=D)    # [D, FF]
        wpjT_v = wpj_gath[:].rearrange("(f o) -> f o", f=FF)   # [FF, D]

        # ---------- Phase D: out-proj (fp8 DoubleRow) + residual + LN2 ----
        mT = main.tile([128, 16, 512], dt.float8e4, tag="bigB", bufs=2,
                       name="mT")
        h2_pool = tc.tile_pool(name="h2a", bufs=4)
        h2a = h2_pool.__enter__()
        h2acc = [h2a.tile([128, D], dt.float32, tag="h2", bufs=4,
                          name=f"h2_{t}") for t in range(4)]
        with tc.tile_pool(name="wo", bufs=3) as wop, \
             tc.tile_pool(name="zps", bufs=1, space="PSUM") as zps:
            for og2 in range(2):
                o_base = og2 * 1024
                pss = [None] * 8
                for g in range(8):          # dj pairs
                    wt = wop.tile([128, 2, 1024], dt.float8e4, tag="wot")
                    nc.sync.dma_start(
                        out=wt[:],
                        in_=bass.AP(tensor=wo_gath.tensor,
                                    offset=wo_gath.offset
                                    + 2 * g * 128 * D + o_base,
                                    ap=[[D, 128], [128 * D, 2], [1, 1024]]))
                    for osub in range(2):
                        for tb in range(4):
                            k = osub * 4 + tb
                            if pss[k] is None:
                                pss[k] = zps.tile([128, 512], dt.float32,
                                                  tag=f"z{k}", name=f"z_{k}")
                            nc.tensor.matmul(
                                pss[k][:],
                                OT[:, 2 * g:2 * g + 2,
                                   tb * 128:(tb + 1) * 128],
                                wt[:, :, osub * 512:(osub + 1) * 512],
                                start=(g == 0), stop=(g == 7),
                                perf_mode=PM.DoubleRow)
                for osub in range(2):
                    for tb in range(4):
                        sl = slice(o_base + osub * 512,
                                   o_base + osub * 512 + 512)
                        nc.vector.tensor_scalar_mul(h2acc[tb][:, sl],
                                                    pss[osub * 4 + tb][:],
                                                    1.0 / WSCALE)
                        nc.vector.tensor_add(h2acc[tb][:, sl],
                                             h2acc[tb][:, sl], bo_bc[:, sl])
        with tc.tile_pool(name="xd", bufs=2) as xd, \
             tc.tile_pool(name="md", bufs=1) as md, \
             tc.tile_pool(name="trps2", bufs=4, space="PSUM") as trps2:
            for tb in range(4):
                for xh in range(2):
                    x_t = xd.tile([128, 1024], dt.float32, tag="x2")
                    nc.sync.dma_start(
                        out=x_t[:],
                        in_=xl_d[tb * 128:(tb + 1) * 128,
                                 xh * 1024:(xh + 1) * 1024])
                    nc.vector.tensor_add(
                        h2acc[tb][:, xh * 1024:(xh + 1) * 1024],
                        h2acc[tb][:, xh * 1024:(xh + 1) * 1024], x_t[:])
                nc.sync.dma_start(out=h2_d[tb * 128:(tb + 1) * 128, :],
                                  in_=h2acc[tb][:])
                m_t = md.tile([128, D], dt.float32, tag="m")
                layernorm(h2acc[tb], m_t, "g2", "b2")
                for dj in range(16):
                    ps = trps2.tile([128, 128], dt.float32, tag="tp2")
                    nc.tensor.transpose(ps[:], m_t[:, dj * 128:(dj + 1) * 128],
                                        ident[:])
                    nc.vector.tensor_copy(mT[:, dj, tb * 128:(tb + 1) * 128],
                                          ps[:])

        h2_pool.__exit__(None, None, None)

        # ---------- Phase E: MLP (fp8 DoubleRow) ----------
        gt_pool = tc.tile_pool(name="gtpl", bufs=1)
        gtpl = gt_pool.__enter__()
        GT1 = gtpl.tile([128, 32, 512], dt.float8e4, name="GT1")
        GT2 = gtpl.tile([128, 32, 512], dt.float8e4, name="GT2")

        def gt_pair(k, c0, c1):
            # lhsT [128, 2, c1-c0] for fti pair (2k, 2k+1)
            if 2 * k < 32:
                return GT1[:, 2 * k:2 * k + 2, c0:c1]
            return GT2[:, 2 * k - 32:2 * k - 30, c0:c1]

        with tc.tile_pool(name="wfc", bufs=6) as wfcp, \
             tc.tile_pool(name="ups", bufs=1, space="PSUM") as ups:
            for FG in range(8):            # 1024 f-cols per group
                pss = [None] * 8
                for g in range(8):         # dj pairs
                    wt = wfcp.tile([128, 2, 1024], dt.float8e4, tag="wfct")
                    nc.sync.dma_start(
                        out=wt[:],
                        in_=bass.AP(tensor=wfc_gath.tensor,
                                    offset=wfc_gath.offset
                                    + 2 * g * 128 * FF + FG * 1024,
                                    ap=[[FF, 128], [128 * FF, 2], [1, 1024]]))
                    for fsub in range(8):
                        if pss[fsub] is None:
                            pss[fsub] = ups.tile([128, 512], dt.float32,
                                                 tag=f"u{fsub}",
                                                 name=f"u_{fsub}")
                        nc.tensor.matmul(
                            pss[fsub][:],
                            wt[:, :, fsub * 128:(fsub + 1) * 128],
                            mT[:, 2 * g:2 * g + 2, :],
                            start=(g == 0), stop=(g == 7),
                            perf_mode=PM.DoubleRow)
                for fsub in range(8):
                    fti = FG * 8 + fsub
                    dst = (GT1[:, fti, 0:512] if fti < 32
                           else GT2[:, fti - 32, 0:512])
                    nc.scalar.activation(dst, pss[fsub][:],
                                         AF.Gelu_apprx_tanh,
                                         bias=bfc_pp[:, fti:fti + 1],
                                         scale=1.0 / WSCALE)
        with tc.tile_pool(name="wpj", bufs=5) as wpjp, \
             tc.tile_pool(name="yps", bufs=1, space="PSUM") as yps, \
             tc.tile_pool(name="outp", bufs=6) as outp:
            for tg in range(2):            # tt groups of 2
                pss = {}
                h2s_g = {}
                for k in range(32):        # ft pairs
                    wt = wpjp.tile([128, 2, D], dt.float8e4, tag="wpjt")
                    nc.sync.dma_start(
                        out=wt[:],
                        in_=bass.AP(tensor=wpj_gath.tensor,
                                    offset=wpj_gath.offset + 2 * k * 128 * D,
                                    ap=[[D, 128], [128 * D, 2], [1, D]]))
                    for ob in range(4):
                        for ti in range(2):
                            tt = tg * 2 + ti
                            key = (ob, ti)
                            if key not in pss:
                                pss[key] = yps.tile(
                                    [128, 512], dt.float32,
                                    tag=f"y{ob}{ti}", name=f"y_{ob}_{ti}")
                            nc.tensor.matmul(
                                pss[key][:],
                                gt_pair(k, tt * 128, (tt + 1) * 128),
                                wt[:, :, ob * 512:(ob + 1) * 512],
                                start=(k == 0), stop=(k == 31),
                                perf_mode=PM.DoubleRow)
                for ti in range(2):
                    tt = tg * 2 + ti
                    h2s = outp.tile([128, D], dt.float32, tag="h2s",
                                    bufs=2, name=f"h2s_{tt}")
                    nc.sync.dma_start(
                        out=h2s[:], in_=h2_d[tt * 128:(tt + 1) * 128, :])
                    h2s_g[ti] = h2s
                for ob in range(4):
                    for ti in range(2):
                        tt = tg * 2 + ti
                        sl = slice(ob * 512, ob * 512 + 512)
                        o_t = outp.tile([128, 512], dt.float32, tag="o")
                        nc.vector.tensor_scalar_mul(o_t[:], pss[(ob, ti)][:],
                                                    1.0 / WSCALE)
                        nc.vector.tensor_add(o_t[:], o_t[:], bpj_bc[:, sl])
                        nc.vector.tensor_add(o_t[:], o_t[:],
                                             h2s_g[ti][:, sl])
                        nc.sync.dma_start(
                            out=out_d[tt * 128:(tt + 1) * 128, sl],
                            in_=o_t[:])
        gt_pool.__exit__(None, None, None)
        stack.close()

    nc.compile()
    return nc


def _host_prep(inputs):
    f32 = lambda k: np.ascontiguousarray(np.asarray(inputs[k], np.float32))
    x = f32("hidden_states")
    wqT = np.ascontiguousarray(f32("wq").T)
    wkT = np.ascontiguousarray(f32("wk").T)
    wvT = np.ascontiguousarray(f32("wv").T)
    woT = np.ascontiguousarray(f32("wo").T).ravel()
    wfcT = np.ascontiguousarray(f32("w_fc").T).ravel()
    wpjT = np.ascontiguousarray(f32("w_proj").T).ravel()
    kp = np.arange(128)
    q_f = np.arange(256)
    in_maps = []
    for c in range(NC):
        mask = np.empty((128, 4, 2, 512), np.float32)
        for jp in range(4):
            for tb in range(2):
                for half in range(2):
                    j = 2 * jp + half
                    ktok = 8 * (128 * tb + kp)[:, None] + j
                    qtok = 8 * q_f[None, :] + c
                    mask[:, jp, tb, half * 256:(half + 1) * 256] = np.where(
                        ktok <= qtok, 1.0, 0.0)
        in_maps.append({
            "xl": np.concatenate([x[0, c::NC, :], x[1, c::NC, :]], 0),
            "wqT": wqT, "wkT": wkT, "wvT": wvT,
            "wo_ch": woT[c * WO_CH:(c + 1) * WO_CH],
            "wfc_ch": wfcT[c * WFC_CH:(c + 1) * WFC_CH],
            "wpj_ch": wpjT[c * WPJ_CH:(c + 1) * WPJ_CH],
            "mask": mask.astype(ml_dtypes.bfloat16),
            "ln1g": f32("ln1_g"), "ln1b": f32("ln1_b"),
            "ln2g": f32("ln2_g"), "ln2b": f32("ln2_b"),
            "bo": f32("bo"), "bfc": f32("b_fc"), "bpj": f32("b_proj"),
        })
    return in_maps


def kernel(**inputs) -> np.ndarray:
    in_maps = _host_prep(inputs)
    key = (not bool(np.all(np.asarray(inputs["ln1_g"]) == 1.0)),
           not bool(np.all(np.asarray(inputs["ln1_b"]) == 0.0)),
           not bool(np.all(np.asarray(inputs["ln2_g"]) == 1.0)),
           not bool(np.all(np.asarray(inputs["ln2_b"]) == 0.0)))
    if key not in _CACHE:
        _CACHE[key] = _build(*key)
    nc = _CACHE[key]
    res = run_bass_kernel_spmd(nc, in_maps, core_ids=list(range(NC)))
    if res.exec_time_ns is not None:
        print(f"HW exec time: {res.exec_time_ns} ns")
    out = np.zeros((B, S, D), np.float32)
    for c in range(NC):
        o = res.results[c]["out"]
        out[0, c::NC] = o[:RPC]
        out[1, c::NC] = o[RPC:]
    return out


# revision 16
# speedup vs baseline: 1.2481x; 1.0678x over previous
"""BinaryGPTNeoBlock on 8 trn2 NeuronCores.

Sequence-parallel over 8 cores: core c owns rows {c, c+8, ...} of both
batch elements (256 per batch, 512 total). Collectives (one stream, in
program order): AllGather K (bf16), V (fp8), then tanh'd+scaled fp8
out-proj/fc/proj weights -- all triggered early so they hide under the
QKV projections and attention. Out-proj and both MLP matmuls run fp8
DoubleRow (weights scaled x1024 into e4m3 range, descaled on PSUM
read); q/k/scores stay bf16 for softmax fidelity. Attention exp reads
PSUM directly and causal masking is a multiplicative bf16 {0,1} mask
applied after exp.

Self-contained: hardcodes shapes; host only shards/transposes/builds masks.
"""

import numpy as np
import ml_dtypes

import concourse.bass as bass
import concourse.tile as tile
from concourse import bacc, mybir
from concourse.bass_utils import run_bass_kernel_spmd
from concourse.masks import make_identity

B, S, D = 2, 2048, 2048
H = 16
HD = 128
FF = 4 * D
EPS = 1e-5
NC = 8
RPC = S // NC          # 256 rows per core per batch
TL = 2 * RPC           # 512 local rows
NKV = TL * D           # elems of K^T (== of V) per core
WO_CH = D * D // NC
WFC_CH = D * FF // NC
WPJ_CH = FF * D // NC
WSCALE = 1024.0        # fp8 weight scale (w in +-0.0221 -> +-22.6)

dt = mybir.dt
AF = mybir.ActivationFunctionType
OP = mybir.AluOpType
PM = mybir.MatmulPerfMode

_CACHE = {}


def _build(apply_g1, apply_b1, apply_g2, apply_b2):
    nc = bacc.Bacc("TRN2", target_bir_lowering=False, debug=False,
                   num_devices=NC)

    xl_d = nc.dram_tensor("xl", [TL, D], dt.float32, kind="ExternalInput").ap()
    wqT_d = nc.dram_tensor("wqT", [D, D], dt.float32, kind="ExternalInput").ap()
    wkT_d = nc.dram_tensor("wkT", [D, D], dt.float32, kind="ExternalInput").ap()
    wvT_d = nc.dram_tensor("wvT", [D, D], dt.float32, kind="ExternalInput").ap()
    wo_ch_d = nc.dram_tensor("wo_ch", [WO_CH], dt.float32,
                             kind="ExternalInput").ap()
    wfc_ch_d = nc.dram_tensor("wfc_ch", [WFC_CH], dt.float32,
                              kind="ExternalInput").ap()
    wpj_ch_d = nc.dram_tensor("wpj_ch", [WPJ_CH], dt.float32,
                              kind="ExternalInput").ap()
    mask_d = nc.dram_tensor("mask", [128, 8, 128], dt.bfloat16,
                            kind="ExternalInput").ap()
    ln1g_d = nc.dram_tensor("ln1g", [D], dt.float32, kind="ExternalInput").ap()
    ln1b_d = nc.dram_tensor("ln1b", [D], dt.float32, kind="ExternalInput").ap()
    ln2g_d = nc.dram_tensor("ln2g", [D], dt.float32, kind="ExternalInput").ap()
    ln2b_d = nc.dram_tensor("ln2b", [D], dt.float32, kind="ExternalInput").ap()
    bo_d = nc.dram_tensor("bo", [D], dt.float32, kind="ExternalInput").ap()
    bfc_d = nc.dram_tensor("bfc", [FF], dt.float32, kind="ExternalInput").ap()
    bpj_d = nc.dram_tensor("bpj", [D], dt.float32, kind="ExternalInput").ap()
    out_d = nc.dram_tensor("out", [TL, D], dt.float32,
                           kind="ExternalOutput").ap()

    def bcast_row(src_ap, n):
        return bass.AP(tensor=src_ap.tensor, offset=src_ap.offset,
                       ap=[[0, 128], [1, n]])

    with tile.TileContext(nc) as tc:
        import contextlib
        stack = contextlib.ExitStack()
        main = stack.enter_context(tc.tile_pool(name="main", bufs=1))
        dram = stack.enter_context(
            tc.tile_pool(name="dram", bufs=1, space="DRAM"))

        ident = main.tile([128, 128], dt.float32)
        make_identity(nc, ident[:])
        ones_col = main.tile([128, 1], dt.float32)
        nc.vector.memset(ones_col[:], 1.0)
        ones_col_b = main.tile([128, 1], dt.bfloat16)
        nc.vector.tensor_copy(ones_col_b[:], ones_col[:])
        ones_row = main.tile([1, 128], dt.float32)
        nc.vector.memset(ones_row[:], 1.0)
        eps_t = main.tile([128, 1], dt.float32)
        nc.vector.memset(eps_t[:], EPS)
        bo_bc = main.tile([128, D], dt.float32)
        nc.sync.dma_start(out=bo_bc[:], in_=bcast_row(bo_d, D))
        bpj_bc = main.tile([128, D], dt.float32)
        nc.sync.dma_start(out=bpj_bc[:], in_=bcast_row(bpj_d, D))
        ln_bc = {}
        for nm, flag, src in (("g1", apply_g1, ln1g_d),
                              ("b1", apply_b1, ln1b_d),
                              ("g2", apply_g2, ln2g_d),
                              ("b2", apply_b2, ln2b_d)):
            if flag:
                t = main.tile([128, D], dt.float32, name=f"ln_{nm}")
                nc.sync.dma_start(out=t[:], in_=bcast_row(src, D))
                ln_bc[nm] = t
        bfc_pp = main.tile([128, FF // 128], dt.float32)
        nc.sync.dma_start(
            out=bfc_pp[:],
            in_=bass.AP(tensor=bfc_d.tensor, offset=bfc_d.offset,
                        ap=[[1, 128], [128, FF // 128]]))
        mask_pool = tc.tile_pool(name="maskp", bufs=1)
        maskp = mask_pool.__enter__()
        masks = maskp.tile([128, 8, 128], dt.bfloat16)
        nc.sync.dma_start(out=masks[:], in_=mask_d[:])

        # Long-lived K/V staging pool (created before phase-B pools so its
        # SBUF doesn't alias them; sized for 2 head-groups of prefetch).
        kvh_pool = tc.tile_pool(name="kvh", bufs=1)
        kvh = kvh_pool.__enter__()

        # big rotating slots: hT -> OT reuse, QT -> mT reuse
        hT = main.tile([128, 16, 512], dt.bfloat16, tag="bigA", bufs=2,
                       name="hT")
        QT = main.tile([128, 16, 512], dt.bfloat16, tag="bigA", bufs=2,
                       name="QT")

        h2_d = dram.tile([TL, D], dt.float32)

        def layernorm(x_t, h_t, gk, bk):
            with tc.tile_pool(name="lnp", bufs=2) as lp:
                st = lp.tile([128, 4, 6], dt.float32, tag="st")
                xr = x_t[:].rearrange("p (n f) -> p n f", n=4)
                for sg in range(4):
                    nc.vector.bn_stats(out=st[:, sg, :], in_=xr[:, sg, :])
                mv = lp.tile([128, 2], dt.float32, tag="mv")
                nc.vector.bn_aggr(out=mv[:], in_=st[:])
                std = lp.tile([128, 1], dt.float32, tag="sd")
                nc.scalar.activation(std[:], mv[:, 1:2], AF.Sqrt,
                                     bias=eps_t[:])
                rstd = lp.tile([128, 1], dt.float32, tag="rs")
                nc.vector.reciprocal(rstd[:], std[:])
                nc.vector.tensor_scalar(h_t[:], x_t[:], mv[:, 0:1], rstd[:],
                                        op0=OP.subtract, op1=OP.mult)
                if gk in ln_bc:
                    nc.vector.tensor_mul(h_t[:], h_t[:], ln_bc[gk][:])
                if bk in ln_bc:
                    nc.vector.tensor_add(h_t[:], h_t[:], ln_bc[bk][:])

        # ---------- Phase A: x -> LN1 -> h^T ----------
        with tc.tile_pool(name="xa", bufs=2) as xa, \
             tc.tile_pool(name="ha", bufs=2) as ha, \
             tc.tile_pool(name="trps", bufs=4, space="PSUM") as trps:
            for tb in range(4):
                x_t = xa.tile([128, D], dt.float32, tag="x")
                nc.sync.dma_start(out=x_t[:],
                                  in_=xl_d[tb * 128:(tb + 1) * 128, :])
                h_t = ha.tile([128, D], dt.float32, tag="h")
                layernorm(x_t, h_t, "g1", "b1")
                for dj in range(16):
                    ps = trps.tile([128, 128], dt.float32, tag="tp")
                    nc.tensor.transpose(ps[:], h_t[:, dj * 128:(dj + 1) * 128],
                                        ident[:])
                    nc.vector.tensor_copy(hT[:, dj, tb * 128:(tb + 1) * 128],
                                          ps[:])

        # wprep: tanh+scale fp8 chunks of wo/wfc/wpj, emitted interleaved
        # into the attention head-group loop so they fill ACT/DMA idle
        # slots there (program-order priority puts them behind attention).
        wo_bounce = dram.tile([WO_CH], dt.float8e4)
        wfc_bounce = dram.tile([WFC_CH], dt.float8e4)
        wpj_bounce = dram.tile([WPJ_CH], dt.float8e4)
        wprep_pool = tc.tile_pool(name="wprep", bufs=2)
        wprep = wprep_pool.__enter__()
        _wprep_jobs = []
        for src, dst, n_t in ((wo_ch_d, wo_bounce, WO_CH // (128 * 2048)),
                              (wfc_ch_d, wfc_bounce, WFC_CH // (128 * 2048)),
                              (wpj_ch_d, wpj_bounce, WPJ_CH // (128 * 2048))):
            for i in range(n_t):
                _wprep_jobs.append((src, dst, i))

        def emit_wprep(n):
            for _ in range(n):
                if not _wprep_jobs:
                    return
                src, dst, i = _wprep_jobs.pop(0)
                raw = wprep.tile([128, 2048], dt.float32, tag="wraw")
                nc.sync.dma_start(
                    out=raw[:],
                    in_=src[i * 128 * 2048:(i + 1) * 128 * 2048]
                    .rearrange("(p f) -> p f", p=128))
                tnh = wprep.tile([128, 2048], dt.bfloat16, tag="wtnh")
                nc.scalar.activation(tnh[:], raw[:], AF.Tanh)
                sc8 = wprep.tile([128, 2048], dt.float8e4, tag="wsc")
                nc.vector.tensor_scalar_mul(sc8[:], tnh[:], WSCALE)
                nc.sync.dma_start(
                    out=dst[i * 128 * 2048:(i + 1) * 128 * 2048]
                    .rearrange("(p f) -> p f", p=128), in_=sc8[:])

        # ---------- Phase B: QKV ----------
        k_bounce = dram.tile([NKV], dt.bfloat16)
        v_bounce = dram.tile([NKV], dt.float8e4)
        k_gath = dram.tile([NC * NKV], dt.bfloat16, addr_space="Shared")
        v_gath = dram.tile([NC * NKV], dt.float8e4, addr_space="Shared")
        wo_gath = dram.tile([NC * WO_CH], dt.float8e4, addr_space="Shared")
        wfc_gath = dram.tile([NC * WFC_CH], dt.float8e4, addr_space="Shared")
        wpj_gath = dram.tile([NC * WPJ_CH], dt.float8e4, addr_space="Shared")

        def project_qk(wT_dram, kind):
            # feature-major output via PE transpose; og(4) x [128,512] loads
            with tc.tile_pool(name=f"pw_{kind}", bufs=4) as wp, \
                 tc.tile_pool(name=f"po_{kind}", bufs=4) as op_, \
                 tc.tile_pool(name=f"pp_{kind}", bufs=1, space="PSUM") as pp, \
                 tc.tile_pool(name=f"pt_{kind}", bufs=4, space="PSUM") as tp2:
                for og in range(4):
                    o_base = og * 512
                    ktacc = []
                    if kind == "k":
                        for k4 in range(4):
                            ka = op_.tile([128, 512], dt.bfloat16, tag="ka",
                                          bufs=8, name=f"ka_{og}_{k4}")
                            ktacc.append(ka)
                    pss = [None] * 4
                    for dj in range(16):
                        raw = wp.tile([128, 512], dt.float32, tag="raw")
                        nc.sync.dma_start(
                            out=raw[:],
                            in_=wT_dram[dj * 128:(dj + 1) * 128,
                                        o_base:o_base + 512])
                        tnh = wp.tile([128, 512], dt.bfloat16, tag="tnh")
                        nc.scalar.activation(tnh[:], raw[:], AF.Tanh)
                        for tb in range(4):
                            if pss[tb] is None:
                                pss[tb] = pp.tile([128, 512], dt.float32,
                                                  tag=f"ps{tb}",
                                                  name=f"ps_{kind}_{tb}")
                            nc.tensor.matmul(
                                pss[tb][:],
                                hT[:, dj, tb * 128:(tb + 1) * 128],
                                tnh[:], start=(dj == 0), stop=(dj == 15))
                    for tb in range(4):
                        tm = op_.tile([128, 512], dt.float32, tag="tm")
                        nc.vector.tensor_copy(tm[:], pss[tb][:])
                        for k4 in range(4):
                            dj2 = (o_base + k4 * 128) // 128
                            ps2 = tp2.tile([128, 128], dt.float32, tag="t2")
                            nc.tensor.transpose(
                                ps2[:], tm[:, k4 * 128:(k4 + 1) * 128],
                                ident[:])
                            if kind == "q":
                                nc.vector.tensor_copy(
                                    QT[:, dj2, tb * 128:(tb + 1) * 128],
                                    ps2[:])
                            else:
                                nc.vector.tensor_copy(
                                    ktacc[k4][:, tb * 128:(tb + 1) * 128],
                                    ps2[:])
                    if kind == "k":
                        for k4 in range(4):
                            dj2 = (o_base + k4 * 128) // 128
                            nc.sync.dma_start(
                                out=k_bounce[dj2 * 128 * TL:
                                             (dj2 + 1) * 128 * TL]
                                .rearrange("(p t) -> p t", p=128),
                                in_=ktacc[k4][:])

        def project_v(wT_dram):
            # token-major; og2(2) x [128,1024] loads; full-row fp8 stores
            with tc.tile_pool(name="pw_v", bufs=4) as wp, \
                 tc.tile_pool(name="po_v", bufs=4) as op_, \
                 tc.tile_pool(name="pp_v", bufs=1, space="PSUM") as pp:
                vacc = [op_.tile([128, D], dt.float8e4, tag="va", bufs=4,
                                 name=f"va_{t}") for t in range(4)]
                for og2 in range(2):
                    o_base = og2 * 1024
                    pss = [None] * 8
                    for dj in range(16):
                        raw = wp.tile([128, 1024], dt.float32, tag="raw")
                        nc.sync.dma_start(
                            out=raw[:],
                            in_=wT_dram[dj * 128:(dj + 1) * 128,
                                        o_base:o_base + 1024])
                        tnh = wp.tile([128, 1024], dt.bfloat16, tag="tnh")
                        nc.scalar.activation(tnh[:], raw[:], AF.Tanh)
                        for osub in range(2):
                            for tb in range(4):
                                k = osub * 4 + tb
                                if pss[k] is None:
                                    pss[k] = pp.tile([128, 512], dt.float32,
                                                     tag=f"ps{k}",
                                                     name=f"ps_v_{k}")
                                nc.tensor.matmul(
                                    pss[k][:],
                                    hT[:, dj, tb * 128:(tb + 1) * 128],
                                    tnh[:, osub * 512:(osub + 1) * 512],
                                    start=(dj == 0), stop=(dj == 15))
                    for osub in range(2):
                        for tb in range(4):
                            sl = slice(o_base + osub * 512,
                                       o_base + osub * 512 + 512)
                            nc.vector.tensor_copy(vacc[tb][:, sl],
                                                  pss[osub * 4 + tb][:])
                for tb in range(4):
                    nc.sync.dma_start(
                        out=v_bounce[tb * 128 * D:(tb + 1) * 128 * D]
                        .rearrange("(p t) -> p t", p=128),
                        in_=vacc[tb][:])

        project_qk(wkT_d, "k")
        nc.gpsimd.collective_compute(
            "AllGather", OP.bypass, replica_groups=[list(range(NC))],
            ins=[k_bounce[:]], outs=[k_gath[:]])
        project_v(wvT_d)
        nc.gpsimd.collective_compute(
            "AllGather", OP.bypass, replica_groups=[list(range(NC))],
            ins=[v_bounce[:]], outs=[v_gath[:]])
        project_qk(wqT_d, "q")

        # ---------- Phase C: attention ----------
        # Causal structure (strided sharding, chunk = 128 partitions of
        # k-tokens): key-half tb=0 (global k < 1024) is fully unmasked for
        # local q-half 1 (global q >= 1024) and diagonal for q-half 0;
        # key-half tb=1 is fully masked for q-half 0 (skipped) and diagonal
        # for q-half 1. The diagonal 128x128 mask is the SAME bf16 {0,1}
        # tile for both halves. exp runs on wide tiles to amortize the
        # ACT per-instruction overhead.
        OT = main.tile([128, 16, 512], dt.float8e4, tag="bigB", bufs=2,
                       name="OT")
        with tc.tile_pool(name="att", bufs=4) as att, \
             tc.tile_pool(name="attsm", bufs=4) as attsm, \
             tc.tile_pool(name="stps", bufs=2, space="PSUM") as stps, \
             tc.tile_pool(name="otps", bufs=2, space="PSUM") as otps, \
             tc.tile_pool(name="dnps", bufs=1, space="PSUM") as dnps, \
             tc.tile_pool(name="bcps", bufs=1, space="PSUM") as bcps:
            for hg in range(4):            # head groups of 4
                kt_g, v_g = [], []
                for j in range(NC):
                    kt = kvh.tile([128, 4, 512], dt.bfloat16, tag="kth",
                                  bufs=12, name=f"kt_{hg}_{j}")
                    nc.sync.dma_start(
                        out=kt[:],
                        in_=bass.AP(tensor=k_gath.tensor,
                                    offset=k_gath.offset + j * NKV
                                    + hg * 4 * 128 * TL,
                                    ap=[[TL, 128], [128 * TL, 4], [1, TL]]))
                    kt_g.append(kt)
                    vt = kvh.tile([128, 4, 512], dt.float8e4, tag="vth",
                                  bufs=12, name=f"vt_{hg}_{j}")
                    nc.sync.dma_start(
                        out=vt[:],
                        in_=bass.AP(tensor=v_gath.tensor,
                                    offset=v_gath.offset + j * NKV
                                    + hg * 4 * 128,
                                    ap=[[D, 128], [128 * D, 4], [1, 512]]))
                    v_g.append(vt)
                for hh in range(4):
                    h = hg * 4 + hh
                    for b in range(2):
                        ot_ps = otps.tile([128, 256], dt.float32, tag="ot")
                        dn_ps = dnps.tile([1, 256], dt.float32, tag="dn")
                        first = True
                        for tb in range(2):
                            for jq in range(2):    # chunk quads
                                st = stps.tile([128, 1024], dt.float32,
                                               tag="st")
                                pt_e = attsm.tile([128, 1024], dt.bfloat16,
                                                  tag="pte")
                                st3 = st[:].rearrange(
                                    "p (c q) -> p c q", c=4)
                                pe3 = pt_e[:].rearrange(
                                    "p (c q) -> p c q", c=4)
                                for jj in range(4):
                                    j = 4 * jq + jj
                                    if tb == 0:
                                        nc.tensor.matmul(
                                            st[:, jj * 256:jj * 256 + 256],
                                            kt_g[j][:, hh,
                                                    b * 256:b * 256 + 128],
                                            QT[:, h, b * 256:(b + 1) * 256],
                                            start=True, stop=True)
                                    else:
                                        nc.tensor.matmul(
                                            st[:, jj * 256 + 128:
                                               jj * 256 + 256],
                                            kt_g[j][:, hh, b * 256 + 128:
                                                    b * 256 + 256],
                                            QT[:, h, b * 256 + 128:
                                               b * 256 + 256],
                                            start=True, stop=True)
                                if tb == 0:
                                    nc.scalar.activation(pt_e[:], st[:],
                                                         AF.Exp)
                                    # diagonal mask on q-half 0 quarters
                                    nc.vector.tensor_mul(
                                        pe3[:, :, 0:128], pe3[:, :, 0:128],
                                        masks[:, 4 * jq:4 * jq + 4, :])
                                else:
                                    nc.scalar.activation(
                                        pe3[:, :, 128:256],
                                        st3[:, :, 128:256], AF.Exp)
                                    nc.vector.tensor_mul(
                                        pe3[:, :, 128:256],
                                        pe3[:, :, 128:256],
                                        masks[:, 4 * jq:4 * jq + 4, :])
                                for jj in range(4):
                                    j = 4 * jq + jj
                                    last = (tb == 1 and jq == 1 and jj == 3)
                                    if tb == 0:
                                        rhs = pt_e[:, jj * 256:jj * 256 + 256]
                                        ot_dst = ot_ps[:]
                                        dn_dst = dn_ps[:]
                                    else:
                                        rhs = pt_e[:, jj * 256 + 128:
                                                   jj * 256 + 256]
                                        ot_dst = ot_ps[:, 128:256]
                                        dn_dst = dn_ps[:, 128:256]
                                    nc.tensor.matmul(
                                        ot_dst,
                                        v_g[j][:, 2 * b + tb,
                                               hh * 128:(hh + 1) * 128],
                                        rhs, start=first, stop=last,
                                        skip_group_check=True)
                                    nc.tensor.matmul(
                                        dn_dst, ones_col_b[:], rhs,
                                        start=first, stop=last,
                                        skip_group_check=True)
                                    first = False
                        dn_sb = att.tile([1, 256], dt.float32, tag="dns")
                        nc.vector.reciprocal(dn_sb[:], dn_ps[:])
                        bc_ps = bcps.tile([128, 256], dt.float32, tag="bc")
                        nc.tensor.matmul(bc_ps[:], ones_row[:], dn_sb[:],
                                         start=True, stop=True)
                        bc_sb = att.tile([128, 256], dt.float32, tag="bcs")
                        nc.vector.tensor_copy(bc_sb[:], bc_ps[:])
                        nc.vector.tensor_mul(OT[:, h, b * 256:(b + 1) * 256],
                                             ot_ps[:], bc_sb[:])
                # interleave fp8 weight prep + gathers into ACT/DMA idle
                if hg == 0:
                    emit_wprep(12)         # all wo chunks + 8 wfc
                    nc.gpsimd.collective_compute(
                        "AllGather", OP.bypass,
                        replica_groups=[list(range(NC))],
                        ins=[wo_bounce[:]], outs=[wo_gath[:]])
                elif hg == 1:
                    emit_wprep(8)          # rest of wfc
                    nc.gpsimd.collective_compute(
                        "AllGather", OP.bypass,
                        replica_groups=[list(range(NC))],
                        ins=[wfc_bounce[:]], outs=[wfc_gath[:]])
                elif hg == 2:
                    emit_wprep(16)         # all wpj chunks
                    nc.gpsimd.collective_compute(
                        "AllGather", OP.bypass,
                        replica_groups=[list(range(NC))],
                        ins=[wpj_bounce[:]], outs=[wpj_gath[:]])

        wprep_pool.__exit__(None, None, None)
        kvh_pool.__exit__(None, None, None)
        mask_pool.__exit__(None, None, None)

        wo_v = wo_gath[:].rearrange("(d o) -> d o", d=D)       # [D, D]
        wfcT_v = wfc_gath[:].rearrange("(d f) -> d f", d=D)    # [D, FF]
        wpjT_v = wpj_gath[:].rearrange("(f o) -> f o", f=FF)   # [FF, D]

        # ---------- Phase D: out-proj (fp8 DoubleRow) + residual + LN2 ----
        mT = main.tile([128, 16, 512], dt.float8e4, tag="bigB", bufs=2,
                       name="mT")
        h2_pool = tc.tile_pool(name="h2a", bufs=4)
        h2a = h2_pool.__enter__()
        h2acc = [h2a.tile([128, D], dt.float32, tag="h2", bufs=4,
                          name=f"h2_{t}") for t in range(4)]
        with tc.tile_pool(name="wo", bufs=3) as wop, \
             tc.tile_pool(name="zps", bufs=1, space="PSUM") as zps:
            for og2 in range(2):
                o_base = og2 * 1024
                pss = [None] * 8
                for g in range(8):          # dj pairs
                    wt = wop.tile([128, 2, 1024], dt.float8e4, tag="wot")
                    nc.sync.dma_start(
                        out=wt[:],
                        in_=bass.AP(tensor=wo_gath.tensor,
                                    offset=wo_gath.offset
                                    + 2 * g * 128 * D + o_base,
                                    ap=[[D, 128], [128 * D, 2], [1, 1024]]))
                    for osub in range(2):
                        for tb in range(4):
                            k = osub * 4 + tb
                            if pss[k] is None:
                                pss[k] = zps.tile([128, 512], dt.float32,
                                                  tag=f"z{k}", name=f"z_{k}")
                            nc.tensor.matmul(
                                pss[k][:],
                                OT[:, 2 * g:2 * g + 2,
                                   tb * 128:(tb + 1) * 128],
                                wt[:, :, osub * 512:(osub + 1) * 512],
                                start=(g == 0), stop=(g == 7),
                                perf_mode=PM.DoubleRow)
                for osub in range(2):
                    for tb in range(4):
                        sl = slice(o_base + osub * 512,
                                   o_base + osub * 512 + 512)
                        nc.vector.tensor_scalar_mul(h2acc[tb][:, sl],
                                                    pss[osub * 4 + tb][:],
                                                    1.0 / WSCALE)
                        nc.vector.tensor_add(h2acc[tb][:, sl],
                                             h2acc[tb][:, sl], bo_bc[:, sl])
        with tc.tile_pool(name="xd", bufs=2) as xd, \
             tc.tile_pool(name="md", bufs=1) as md, \
             tc.tile_pool(name="trps2", bufs=4, space="PSUM") as trps2:
            for tb in range(4):
                for xh in range(2):
                    x_t = xd.tile([128, 1024], dt.float32, tag="x2")
                    nc.sync.dma_start(
                        out=x_t[:],
                        in_=xl_d[tb * 128:(tb + 1) * 128,
                                 xh * 1024:(xh + 1) * 1024])
                    nc.vector.tensor_add(
                        h2acc[tb][:, xh * 1024:(xh + 1) * 1024],
                        h2acc[tb][:, xh * 1024:(xh + 1) * 1024], x_t[:])
                nc.sync.dma_start(out=h2_d[tb * 128:(tb + 1) * 128, :],
                                  in_=h2acc[tb][:])
                m_t = md.tile([128, D], dt.float32, tag="m")
                layernorm(h2acc[tb], m_t, "g2", "b2")
                for dj in range(16):
                    ps = trps2.tile([128, 128], dt.float32, tag="tp2")
                    nc.tensor.transpose(ps[:], m_t[:, dj * 128:(dj + 1) * 128],
                                        ident[:])
                    nc.vector.tensor_copy(mT[:, dj, tb * 128:(tb + 1) * 128],
                                          ps[:])

        h2_pool.__exit__(None, None, None)

        # ---------- Phase E: MLP (fp8 DoubleRow) ----------
        gt_pool = tc.tile_pool(name="gtpl", bufs=1)
        gtpl = gt_pool.__enter__()
        GT1 = gtpl.tile([128, 32, 512], dt.float8e4, name="GT1")
        GT2 = gtpl.tile([128, 32, 512], dt.float8e4, name="GT2")

        def gt_pair(k, c0, c1):
            # lhsT [128, 2, c1-c0] for fti pair (2k, 2k+1)
            if 2 * k < 32:
                return GT1[:, 2 * k:2 * k + 2, c0:c1]
            return GT2[:, 2 * k - 32:2 * k - 30, c0:c1]

        with tc.tile_pool(name="wfc", bufs=6) as wfcp, \
             tc.tile_pool(name="ups", bufs=1, space="PSUM") as ups:
            for FG in range(8):            # 1024 f-cols per group
                pss = [None] * 8
                for g in range(8):         # dj pairs
                    wt = wfcp.tile([128, 2, 1024], dt.float8e4, tag="wfct")
                    nc.sync.dma_start(
                        out=wt[:],
                        in_=bass.AP(tensor=wfc_gath.tensor,
                                    offset=wfc_gath.offset
                                    + 2 * g * 128 * FF + FG * 1024,
                                    ap=[[FF, 128], [128 * FF, 2], [1, 1024]]))
                    for fsub in range(8):
                        if pss[fsub] is None:
                            pss[fsub] = ups.tile([128, 512], dt.float32,
                                                 tag=f"u{fsub}",
                                                 name=f"u_{fsub}")
                        nc.tensor.matmul(
                            pss[fsub][:],
                            wt[:, :, fsub * 128:(fsub + 1) * 128],
                            mT[:, 2 * g:2 * g + 2, :],
                            start=(g == 0), stop=(g == 7),
                            perf_mode=PM.DoubleRow)
                for fsub in range(8):
                    fti = FG * 8 + fsub
                    dst = (GT1[:, fti, 0:512] if fti < 32
                           else GT2[:, fti - 32, 0:512])
                    nc.scalar.activation(dst, pss[fsub][:],
                                         AF.Gelu_apprx_tanh,
                                         bias=bfc_pp[:, fti:fti + 1],
                                         scale=1.0 / WSCALE)
        with tc.tile_pool(name="wpj", bufs=5) as wpjp, \
             tc.tile_pool(name="yps", bufs=1, space="PSUM") as yps, \
             tc.tile_pool(name="outp", bufs=6) as outp:
            for tg in range(2):            # tt groups of 2
                pss = {}
                h2s_g = {}
                for k in range(32):        # ft pairs
                    wt = wpjp.tile([128, 2, D], dt.float8e4, tag="wpjt")
                    nc.sync.dma_start(
                        out=wt[:],
                        in_=bass.AP(tensor=wpj_gath.tensor,
                                    offset=wpj_gath.offset + 2 * k * 128 * D,
                                    ap=[[D, 128], [128 * D, 2], [1, D]]))
                    for ob in range(4):
                        for ti in range(2):
                            tt = tg * 2 + ti
                            key = (ob, ti)
                            if key not in pss:
                                pss[key] = yps.tile(
                                    [128, 512], dt.float32,
                                    tag=f"y{ob}{ti}", name=f"y_{ob}_{ti}")
                            nc.tensor.matmul(
                                pss[key][:],
                                gt_pair(k, tt * 128, (tt + 1) * 128),
                                wt[:, :, ob * 512:(ob + 1) * 512],
                                start=(k == 0), stop=(k == 31),
                                perf_mode=PM.DoubleRow)
                for ti in range(2):
                    tt = tg * 2 + ti
                    h2s = outp.tile([128, D], dt.float32, tag="h2s",
                                    bufs=2, name=f"h2s_{tt}")
                    nc.sync.dma_start(
                        out=h2s[:], in_=h2_d[tt * 128:(tt + 1) * 128, :])
                    h2s_g[ti] = h2s
                for ob in range(4):
                    for ti in range(2):
                        tt = tg * 2 + ti
                        sl = slice(ob * 512, ob * 512 + 512)
                        o_t = outp.tile([128, 512], dt.float32, tag="o")
                        nc.vector.tensor_scalar_mul(o_t[:], pss[(ob, ti)][:],
                                                    1.0 / WSCALE)
                        nc.vector.tensor_add(o_t[:], o_t[:], bpj_bc[:, sl])
                        nc.vector.tensor_add(o_t[:], o_t[:],
                                             h2s_g[ti][:, sl])
                        nc.sync.dma_start(
                            out=out_d[tt * 128:(tt + 1) * 128, sl],
                            in_=o_t[:])
        gt_pool.__exit__(None, None, None)
        stack.close()

    nc.compile()
    return nc


def _host_prep(inputs):
    f32 = lambda k: np.ascontiguousarray(np.asarray(inputs[k], np.float32))
    x = f32("hidden_states")
    wqT = np.ascontiguousarray(f32("wq").T)
    wkT = np.ascontiguousarray(f32("wk").T)
    wvT = np.ascontiguousarray(f32("wv").T)
    woT = np.ascontiguousarray(f32("wo").T).ravel()
    wfcT = np.ascontiguousarray(f32("w_fc").T).ravel()
    wpjT = np.ascontiguousarray(f32("w_proj").T).ravel()
    kp = np.arange(128)
    qq = np.arange(128)
    in_maps = []
    for c in range(NC):
        mask = np.empty((128, 8, 128), np.float32)
        for j in range(8):
            mask[:, j, :] = np.where(
                8 * kp[:, None] + j <= 8 * qq[None, :] + c, 1.0, 0.0)
        in_maps.append({
            "xl": np.concatenate([x[0, c::NC, :], x[1, c::NC, :]], 0),
            "wqT": wqT, "wkT": wkT, "wvT": wvT,
            "wo_ch": woT[c * WO_CH:(c + 1) * WO_CH],
            "wfc_ch": wfcT[c * WFC_CH:(c + 1) * WFC_CH],
            "wpj_ch": wpjT[c * WPJ_CH:(c + 1) * WPJ_CH],
            "mask": mask.astype(ml_dtypes.bfloat16),
            "ln1g": f32("ln1_g"), "ln1b": f32("ln1_b"),
            "ln2g": f32("ln2_g"), "ln2b": f32("ln2_b"),
            "bo": f32("bo"), "bfc": f32("b_fc"), "bpj": f32("b_proj"),
        })
    return in_maps


def kernel(**inputs) -> np.ndarray:
    in_maps = _host_prep(inputs)
    key = (not bool(np.all(np.asarray(inputs["ln1_g"]) == 1.0)),
           not bool(np.all(np.asarray(inputs["ln1_b"]) == 0.0)),
           not bool(np.all(np.asarray(inputs["ln2_g"]) == 1.0)),
           not bool(np.all(np.asarray(inputs["ln2_b"]) == 0.0)))
    if key not in _CACHE:
        _CACHE[key] = _build(*key)
    nc = _CACHE[key]
    res = run_bass_kernel_spmd(nc, in_maps, core_ids=list(range(NC)))
    if res.exec_time_ns is not None:
        print(f"HW exec time: {res.exec_time_ns} ns")
    out = np.zeros((B, S, D), np.float32)
    for c in range(NC):
        o = res.results[c]["out"]
        out[0, c::NC] = o[:RPC]
        out[1, c::NC] = o[RPC:]
    return out


# revision 20
# speedup vs baseline: 1.2523x; 1.0034x over previous
"""BinaryGPTNeoBlock on 8 trn2 NeuronCores.

Sequence-parallel over 8 cores: core c owns rows {c, c+8, ...} of both
batch elements (256 per batch, 512 total). Collectives (one stream, in
program order): AllGather K (bf16), V (fp8), then tanh'd+scaled fp8
out-proj/fc/proj weights -- all triggered early so they hide under the
QKV projections and attention. Out-proj and both MLP matmuls run fp8
DoubleRow (weights scaled x1024 into e4m3 range, descaled on PSUM
read); q/k/scores stay bf16 for softmax fidelity. Attention exp reads
PSUM directly and causal masking is a multiplicative bf16 {0,1} mask
applied after exp.

Self-contained: hardcodes shapes; host only shards/transposes/builds masks.
"""

import numpy as np
import ml_dtypes

import concourse.bass as bass
import concourse.tile as tile
from concourse import bacc, mybir
from concourse.bass_utils import run_bass_kernel_spmd
from concourse.masks import make_identity

B, S, D = 2, 2048, 2048
H = 16
HD = 128
FF = 4 * D
EPS = 1e-5
NC = 8
RPC = S // NC          # 256 rows per core per batch
TL = 2 * RPC           # 512 local rows
NKV = TL * D           # elems of K^T (== of V) per core
WO_CH = D * D // NC
WFC_CH = D * FF // NC
WPJ_CH = FF * D // NC
WSCALE = 1024.0        # fp8 weight scale (w in +-0.0221 -> +-22.6)

dt = mybir.dt
AF = mybir.ActivationFunctionType
OP = mybir.AluOpType
PM = mybir.MatmulPerfMode

_CACHE = {}


def _build(apply_g1, apply_b1, apply_g2, apply_b2):
    nc = bacc.Bacc("TRN2", target_bir_lowering=False, debug=False,
                   num_devices=NC)

    xl_d = nc.dram_tensor("xl", [TL, D], dt.float32, kind="ExternalInput").ap()
    wqT_d = nc.dram_tensor("wqT", [D, D], dt.float32, kind="ExternalInput").ap()
    wkT_d = nc.dram_tensor("wkT", [D, D], dt.float32, kind="ExternalInput").ap()
    wvT_d = nc.dram_tensor("wvT", [D, D], dt.float32, kind="ExternalInput").ap()
    wo_ch_d = nc.dram_tensor("wo_ch", [WO_CH], dt.float32,
                             kind="ExternalInput").ap()
    wfc_ch_d = nc.dram_tensor("wfc_ch", [WFC_CH], dt.float32,
                              kind="ExternalInput").ap()
    wpj_ch_d = nc.dram_tensor("wpj_ch", [WPJ_CH], dt.float32,
                              kind="ExternalInput").ap()
    mask_d = nc.dram_tensor("mask", [128, 8, 128], dt.bfloat16,
                            kind="ExternalInput").ap()
    ln1g_d = nc.dram_tensor("ln1g", [D], dt.float32, kind="ExternalInput").ap()
    ln1b_d = nc.dram_tensor("ln1b", [D], dt.float32, kind="ExternalInput").ap()
    ln2g_d = nc.dram_tensor("ln2g", [D], dt.float32, kind="ExternalInput").ap()
    ln2b_d = nc.dram_tensor("ln2b", [D], dt.float32, kind="ExternalInput").ap()
    bo_d = nc.dram_tensor("bo", [D], dt.float32, kind="ExternalInput").ap()
    bfc_d = nc.dram_tensor("bfc", [FF], dt.float32, kind="ExternalInput").ap()
    bpj_d = nc.dram_tensor("bpj", [D], dt.float32, kind="ExternalInput").ap()
    out_d = nc.dram_tensor("out", [TL, D], dt.float32,
                           kind="ExternalOutput").ap()

    def bcast_row(src_ap, n):
        return bass.AP(tensor=src_ap.tensor, offset=src_ap.offset,
                       ap=[[0, 128], [1, n]])

    with tile.TileContext(nc) as tc:
        import contextlib
        stack = contextlib.ExitStack()
        main = stack.enter_context(tc.tile_pool(name="main", bufs=1))
        dram = stack.enter_context(
            tc.tile_pool(name="dram", bufs=1, space="DRAM"))

        ident = main.tile([128, 128], dt.float32)
        make_identity(nc, ident[:])
        ones_col = main.tile([128, 1], dt.float32)
        nc.vector.memset(ones_col[:], 1.0)
        ones_col_b = main.tile([128, 1], dt.bfloat16)
        nc.vector.tensor_copy(ones_col_b[:], ones_col[:])
        ones_row = main.tile([1, 128], dt.float32)
        nc.vector.memset(ones_row[:], 1.0)
        eps_t = main.tile([128, 1], dt.float32)
        nc.vector.memset(eps_t[:], EPS)
        bo_bc = main.tile([128, D], dt.float32)
        nc.sync.dma_start(out=bo_bc[:], in_=bcast_row(bo_d, D))
        bpj_bc = main.tile([128, D], dt.float32)
        nc.sync.dma_start(out=bpj_bc[:], in_=bcast_row(bpj_d, D))
        ln_bc = {}
        for nm, flag, src in (("g1", apply_g1, ln1g_d),
                              ("b1", apply_b1, ln1b_d),
                              ("g2", apply_g2, ln2g_d),
                              ("b2", apply_b2, ln2b_d)):
            if flag:
                t = main.tile([128, D], dt.float32, name=f"ln_{nm}")
                nc.sync.dma_start(out=t[:], in_=bcast_row(src, D))
                ln_bc[nm] = t
        bfc_pp = main.tile([128, FF // 128], dt.float32)
        nc.sync.dma_start(
            out=bfc_pp[:],
            in_=bass.AP(tensor=bfc_d.tensor, offset=bfc_d.offset,
                        ap=[[1, 128], [128, FF // 128]]))
        mask_pool = tc.tile_pool(name="maskp", bufs=1)
        maskp = mask_pool.__enter__()
        masks = maskp.tile([128, 8, 128], dt.bfloat16)
        nc.sync.dma_start(out=masks[:], in_=mask_d[:])

        # Long-lived K/V staging pool (created before phase-B pools so its
        # SBUF doesn't alias them; sized for 2 head-groups of prefetch).
        kvh_pool = tc.tile_pool(name="kvh", bufs=1)
        kvh = kvh_pool.__enter__()

        # big rotating slots: hT -> OT reuse, QT -> mT reuse
        hT = main.tile([128, 16, 512], dt.bfloat16, tag="bigA", bufs=2,
                       name="hT")
        QT = main.tile([128, 16, 512], dt.bfloat16, tag="bigA", bufs=2,
                       name="QT")

        h2_d = dram.tile([TL, D], dt.float32)

        def layernorm(x_t, h_t, gk, bk):
            with tc.tile_pool(name="lnp", bufs=2) as lp:
                st = lp.tile([128, 4, 6], dt.float32, tag="st")
                xr = x_t[:].rearrange("p (n f) -> p n f", n=4)
                for sg in range(4):
                    nc.vector.bn_stats(out=st[:, sg, :], in_=xr[:, sg, :])
                mv = lp.tile([128, 2], dt.float32, tag="mv")
                nc.vector.bn_aggr(out=mv[:], in_=st[:])
                std = lp.tile([128, 1], dt.float32, tag="sd")
                nc.scalar.activation(std[:], mv[:, 1:2], AF.Sqrt,
                                     bias=eps_t[:])
                rstd = lp.tile([128, 1], dt.float32, tag="rs")
                nc.vector.reciprocal(rstd[:], std[:])
                nc.vector.tensor_scalar(h_t[:], x_t[:], mv[:, 0:1], rstd[:],
                                        op0=OP.subtract, op1=OP.mult)
                if gk in ln_bc:
                    nc.vector.tensor_mul(h_t[:], h_t[:], ln_bc[gk][:])
                if bk in ln_bc:
                    nc.vector.tensor_add(h_t[:], h_t[:], ln_bc[bk][:])

        # ---------- Phase A: x -> LN1 -> h^T ----------
        with tc.tile_pool(name="xa", bufs=2) as xa, \
             tc.tile_pool(name="ha", bufs=2) as ha, \
             tc.tile_pool(name="trps", bufs=4, space="PSUM") as trps:
            for tb in range(4):
                x_t = xa.tile([128, D], dt.float32, tag="x")
                nc.sync.dma_start(out=x_t[:],
                                  in_=xl_d[tb * 128:(tb + 1) * 128, :])
                h_t = ha.tile([128, D], dt.float32, tag="h")
                layernorm(x_t, h_t, "g1", "b1")
                for dj in range(16):
                    ps = trps.tile([128, 128], dt.float32, tag="tp")
                    nc.tensor.transpose(ps[:], h_t[:, dj * 128:(dj + 1) * 128],
                                        ident[:])
                    nc.vector.tensor_copy(hT[:, dj, tb * 128:(tb + 1) * 128],
                                          ps[:])

        # wprep: tanh+scale fp8 chunks of wo/wfc/wpj, emitted interleaved
        # into the attention head-group loop so they fill ACT/DMA idle
        # slots there (program-order priority puts them behind attention).
        wo_bounce = dram.tile([WO_CH], dt.float8e4)
        wfc_bounce = dram.tile([WFC_CH], dt.float8e4)
        wpj_bounce = dram.tile([WPJ_CH], dt.float8e4)
        wprep_pool = tc.tile_pool(name="wprep", bufs=2)
        wprep = wprep_pool.__enter__()
        _wprep_jobs = []
        for src, dst, n_t in ((wo_ch_d, wo_bounce, WO_CH // (128 * 2048)),
                              (wfc_ch_d, wfc_bounce, WFC_CH // (128 * 2048)),
                              (wpj_ch_d, wpj_bounce, WPJ_CH // (128 * 2048))):
            for i in range(n_t):
                _wprep_jobs.append((src, dst, i, i == n_t - 1))

        # gate tile: exactly 1.0, but data-dependent on the V gather output.
        # Multiplying the FINAL chunk of each fp8 weight bounce by it makes
        # the wo/wfc/wpj AllGather triggers depend on the V gather, pinning
        # the serial collective queue to the order K, V, wo, wfc, wpj.
        gate = [None]

        def emit_wprep(n):
            for _ in range(n):
                if not _wprep_jobs:
                    return
                src, dst, i, is_last = _wprep_jobs.pop(0)
                raw = wprep.tile([128, 2048], dt.float32, tag="wraw")
                nc.sync.dma_start(
                    out=raw[:],
                    in_=src[i * 128 * 2048:(i + 1) * 128 * 2048]
                    .rearrange("(p f) -> p f", p=128))
                tnh = wprep.tile([128, 2048], dt.bfloat16, tag="wtnh")
                nc.scalar.activation(tnh[:], raw[:], AF.Tanh)
                sc8 = wprep.tile([128, 2048], dt.float8e4, tag="wsc")
                if is_last:
                    nc.vector.tensor_scalar(sc8[:], tnh[:], gate[0][:],
                                            WSCALE, op0=OP.mult, op1=OP.mult)
                else:
                    nc.vector.tensor_scalar_mul(sc8[:], tnh[:], WSCALE)
                nc.sync.dma_start(
                    out=dst[i * 128 * 2048:(i + 1) * 128 * 2048]
                    .rearrange("(p f) -> p f", p=128), in_=sc8[:])

        # ---------- Phase B: QKV ----------
        k_bounce = dram.tile([NKV], dt.bfloat16)
        v_bounce = dram.tile([NKV], dt.float8e4)
        k_gath = dram.tile([NC * NKV], dt.bfloat16, addr_space="Shared")
        v_gath = dram.tile([NC * NKV], dt.float8e4, addr_space="Shared")
        wo_gath = dram.tile([NC * WO_CH], dt.float8e4, addr_space="Shared")
        wfc_gath = dram.tile([NC * WFC_CH], dt.float8e4, addr_space="Shared")
        wpj_gath = dram.tile([NC * WPJ_CH], dt.float8e4, addr_space="Shared")

        def project_qk(wT_dram, kind):
            # feature-major output via PE transpose; og(4) x [128,512] loads
            with tc.tile_pool(name=f"pw_{kind}", bufs=4) as wp, \
                 tc.tile_pool(name=f"po_{kind}", bufs=4) as op_, \
                 tc.tile_pool(name=f"pp_{kind}", bufs=1, space="PSUM") as pp, \
                 tc.tile_pool(name=f"pt_{kind}", bufs=4, space="PSUM") as tp2:
                for og in range(4):
                    o_base = og * 512
                    ktacc = []
                    if kind == "k":
                        for k4 in range(4):
                            ka = op_.tile([128, 512], dt.bfloat16, tag="ka",
                                          bufs=8, name=f"ka_{og}_{k4}")
                            ktacc.append(ka)
                    pss = [None] * 4
                    for dj in range(16):
                        raw = wp.tile([128, 512], dt.float32, tag="raw")
                        nc.sync.dma_start(
                            out=raw[:],
                            in_=wT_dram[dj * 128:(dj + 1) * 128,
                                        o_base:o_base + 512])
                        tnh = wp.tile([128, 512], dt.bfloat16, tag="tnh")
                        nc.scalar.activation(tnh[:], raw[:], AF.Tanh)
                        for tb in range(4):
                            if pss[tb] is None:
                                pss[tb] = pp.tile([128, 512], dt.float32,
                                                  tag=f"ps{tb}",
                                                  name=f"ps_{kind}_{tb}")
                            nc.tensor.matmul(
                                pss[tb][:],
                                hT[:, dj, tb * 128:(tb + 1) * 128],
                                tnh[:], start=(dj == 0), stop=(dj == 15))
                    for tb in range(4):
                        tm = op_.tile([128, 512], dt.float32, tag="tm")
                        nc.vector.tensor_copy(tm[:], pss[tb][:])
                        for k4 in range(4):
                            dj2 = (o_base + k4 * 128) // 128
                            ps2 = tp2.tile([128, 128], dt.float32, tag="t2")
                            nc.tensor.transpose(
                                ps2[:], tm[:, k4 * 128:(k4 + 1) * 128],
                                ident[:])
                            if kind == "q":
                                nc.vector.tensor_copy(
                                    QT[:, dj2, tb * 128:(tb + 1) * 128],
                                    ps2[:])
                            else:
                                nc.vector.tensor_copy(
                                    ktacc[k4][:, tb * 128:(tb + 1) * 128],
                                    ps2[:])
                    if kind == "k":
                        for k4 in range(4):
                            dj2 = (o_base + k4 * 128) // 128
                            nc.sync.dma_start(
                                out=k_bounce[dj2 * 128 * TL:
                                             (dj2 + 1) * 128 * TL]
                                .rearrange("(p t) -> p t", p=128),
                                in_=ktacc[k4][:])

        def project_v(wT_dram):
            # token-major; og2(2) x [128,1024] loads; full-row fp8 stores
            with tc.tile_pool(name="pw_v", bufs=4) as wp, \
                 tc.tile_pool(name="po_v", bufs=4) as op_, \
                 tc.tile_pool(name="pp_v", bufs=1, space="PSUM") as pp:
                vacc = [op_.tile([128, D], dt.float8e4, tag="va", bufs=4,
                                 name=f"va_{t}") for t in range(4)]
                for og2 in range(2):
                    o_base = og2 * 1024
                    pss = [None] * 8
                    for dj in range(16):
                        raw = wp.tile([128, 1024], dt.float32, tag="raw")
                        nc.sync.dma_start(
                            out=raw[:],
                            in_=wT_dram[dj * 128:(dj + 1) * 128,
                                        o_base:o_base + 1024])
                        tnh = wp.tile([128, 1024], dt.bfloat16, tag="tnh")
                        nc.scalar.activation(tnh[:], raw[:], AF.Tanh)
                        for osub in range(2):
                            for tb in range(4):
                                k = osub * 4 + tb
                                if pss[k] is None:
                                    pss[k] = pp.tile([128, 512], dt.float32,
                                                     tag=f"ps{k}",
                                                     name=f"ps_v_{k}")
                                nc.tensor.matmul(
                                    pss[k][:],
                                    hT[:, dj, tb * 128:(tb + 1) * 128],
                                    tnh[:, osub * 512:(osub + 1) * 512],
                                    start=(dj == 0), stop=(dj == 15))
                    for osub in range(2):
                        for tb in range(4):
                            sl = slice(o_base + osub * 512,
                                       o_base + osub * 512 + 512)
                            nc.vector.tensor_copy(vacc[tb][:, sl],
                                                  pss[osub * 4 + tb][:])
                for tb in range(4):
                    nc.sync.dma_start(
                        out=v_bounce[tb * 128 * D:(tb + 1) * 128 * D]
                        .rearrange("(p t) -> p t", p=128),
                        in_=vacc[tb][:])

        project_qk(wkT_d, "k")
        nc.gpsimd.collective_compute(
            "AllGather", OP.bypass, replica_groups=[list(range(NC))],
            ins=[k_bounce[:]], outs=[k_gath[:]])
        project_v(wvT_d)
        nc.gpsimd.collective_compute(
            "AllGather", OP.bypass, replica_groups=[list(range(NC))],
            ins=[v_bounce[:]], outs=[v_gath[:]])
        gate_src = main.tile([128, 1], dt.float8e4, name="gate_src")
        nc.sync.dma_start(
            out=gate_src[:],
            in_=bass.AP(tensor=v_gath.tensor, offset=v_gath.offset,
                        ap=[[1, 128], [1, 1]]))
        gate_t = main.tile([128, 1], dt.float32, name="gate_t")
        nc.vector.tensor_scalar(gate_t[:], gate_src[:], 0.0, 1.0,
                                op0=OP.mult, op1=OP.add)
        gate[0] = gate_t
        project_qk(wqT_d, "q")

        # ---------- Phase C: attention ----------
        # Causal structure (strided sharding, chunk = 128 partitions of
        # k-tokens): key-half tb=0 (global k < 1024) is fully unmasked for
        # local q-half 1 (global q >= 1024) and diagonal for q-half 0;
        # key-half tb=1 is fully masked for q-half 0 (skipped) and diagonal
        # for q-half 1. The diagonal 128x128 mask is the SAME bf16 {0,1}
        # tile for both halves. exp runs on wide tiles to amortize the
        # ACT per-instruction overhead.
        OT = main.tile([128, 16, 512], dt.float8e4, tag="bigB", bufs=2,
                       name="OT")
        with tc.tile_pool(name="att", bufs=4) as att, \
             tc.tile_pool(name="attsm", bufs=4) as attsm, \
             tc.tile_pool(name="stps", bufs=2, space="PSUM") as stps, \
             tc.tile_pool(name="otps", bufs=2, space="PSUM") as otps, \
             tc.tile_pool(name="dnps", bufs=1, space="PSUM") as dnps, \
             tc.tile_pool(name="bcps", bufs=1, space="PSUM") as bcps:
            for hg in range(4):            # head groups of 4
                kt_g, v_g = [], []
                for j in range(NC):
                    kt = kvh.tile([128, 4, 512], dt.bfloat16, tag="kth",
                                  bufs=12, name=f"kt_{hg}_{j}")
                    nc.sync.dma_start(
                        out=kt[:],
                        in_=bass.AP(tensor=k_gath.tensor,
                                    offset=k_gath.offset + j * NKV
                                    + hg * 4 * 128 * TL,
                                    ap=[[TL, 128], [128 * TL, 4], [1, TL]]))
                    kt_g.append(kt)
                    vt = kvh.tile([128, 4, 512], dt.float8e4, tag="vth",
                                  bufs=12, name=f"vt_{hg}_{j}")
                    nc.sync.dma_start(
                        out=vt[:],
                        in_=bass.AP(tensor=v_gath.tensor,
                                    offset=v_gath.offset + j * NKV
                                    + hg * 4 * 128,
                                    ap=[[D, 128], [128 * D, 4], [1, 512]]))
                    v_g.append(vt)
                for hh in range(4):
                    h = hg * 4 + hh
                    # both batches accumulate into one shared PSUM pair
                    # (disjoint column ranges; per-element has_written
                    # handles the staggered starts) for a deeper
                    # independent MM pipeline per head.
                    ot_ps = otps.tile([128, 512], dt.float32, tag="ot")
                    dn_ps = dnps.tile([1, 512], dt.float32, tag="dn")
                    first = True
                    for b in range(2):
                        for tb in range(2):
                            for jq in range(2):    # chunk quads
                                st = stps.tile([128, 1024], dt.float32,
                                               tag="st")
                                pt_e = attsm.tile([128, 1024], dt.bfloat16,
                                                  tag="pte")
                                st3 = st[:].rearrange(
                                    "p (c q) -> p c q", c=4)
                                pe3 = pt_e[:].rearrange(
                                    "p (c q) -> p c q", c=4)
                                for jj in range(4):
                                    j = 4 * jq + jj
                                    if tb == 0:
                                        nc.tensor.matmul(
                                            st[:, jj * 256:jj * 256 + 256],
                                            kt_g[j][:, hh,
                                                    b * 256:b * 256 + 128],
                                            QT[:, h, b * 256:(b + 1) * 256],
                                            start=True, stop=True)
                                    else:
                                        nc.tensor.matmul(
                                            st[:, jj * 256 + 128:
                                               jj * 256 + 256],
                                            kt_g[j][:, hh, b * 256 + 128:
                                                    b * 256 + 256],
                                            QT[:, h, b * 256 + 128:
                                               b * 256 + 256],
                                            start=True, stop=True)
                                if tb == 0:
                                    nc.scalar.activation(pt_e[:], st[:],
                                                         AF.Exp)
                                    # diagonal mask on q-half 0 quarters
                                    nc.vector.tensor_mul(
                                        pe3[:, :, 0:128], pe3[:, :, 0:128],
                                        masks[:, 4 * jq:4 * jq + 4, :])
                                else:
                                    nc.scalar.activation(
                                        pe3[:, :, 128:256],
                                        st3[:, :, 128:256], AF.Exp)
                                    nc.vector.tensor_mul(
                                        pe3[:, :, 128:256],
                                        pe3[:, :, 128:256],
                                        masks[:, 4 * jq:4 * jq + 4, :])
                                for jj in range(4):
                                    j = 4 * jq + jj
                                    last = (b == 1 and tb == 1 and jq == 1
                                            and jj == 3)
                                    if tb == 0:
                                        rhs = pt_e[:, jj * 256:jj * 256 + 256]
                                        ot_dst = ot_ps[:, b * 256:
                                                       b * 256 + 256]
                                        dn_dst = dn_ps[:, b * 256:
                                                       b * 256 + 256]
                                    else:
                                        rhs = pt_e[:, jj * 256 + 128:
                                                   jj * 256 + 256]
                                        ot_dst = ot_ps[:, b * 256 + 128:
                                                       b * 256 + 256]
                                        dn_dst = dn_ps[:, b * 256 + 128:
                                                       b * 256 + 256]
                                    nc.tensor.matmul(
                                        ot_dst,
                                        v_g[j][:, 2 * b + tb,
                                               hh * 128:(hh + 1) * 128],
                                        rhs, start=first, stop=last,
                                        skip_group_check=True)
                                    nc.tensor.matmul(
                                        dn_dst, ones_col_b[:], rhs,
                                        start=first, stop=last,
                                        skip_group_check=True)
                                    first = False
                    dn_sb = att.tile([1, 512], dt.float32, tag="dns")
                    nc.vector.reciprocal(dn_sb[:], dn_ps[:])
                    bc_ps = bcps.tile([128, 512], dt.float32, tag="bc")
                    nc.tensor.matmul(bc_ps[:], ones_row[:], dn_sb[:],
                                     start=True, stop=True)
                    bc_sb = att.tile([128, 512], dt.float32, tag="bcs")
                    nc.vector.tensor_copy(bc_sb[:], bc_ps[:])
                    nc.vector.tensor_mul(OT[:, h, :], ot_ps[:], bc_sb[:])
                # interleave fp8 weight prep + gathers into ACT/DMA idle
                if hg == 0:
                    emit_wprep(12)         # all wo chunks + 8 wfc
                    nc.gpsimd.collective_compute(
                        "AllGather", OP.bypass,
                        replica_groups=[list(range(NC))],
                        ins=[wo_bounce[:]], outs=[wo_gath[:]])
                elif hg == 1:
                    emit_wprep(8)          # rest of wfc
                    nc.gpsimd.collective_compute(
                        "AllGather", OP.bypass,
                        replica_groups=[list(range(NC))],
                        ins=[wfc_bounce[:]], outs=[wfc_gath[:]])
                elif hg == 2:
                    emit_wprep(16)         # all wpj chunks
                    nc.gpsimd.collective_compute(
                        "AllGather", OP.bypass,
                        replica_groups=[list(range(NC))],
                        ins=[wpj_bounce[:]], outs=[wpj_gath[:]])

        wprep_pool.__exit__(None, None, None)
        kvh_pool.__exit__(None, None, None)
        mask_pool.__exit__(None, None, None)

        wo_v = wo_gath[:].rearrange("(d o) -> d o", d=D)       # [D, D]
        wfcT_v = wfc_gath[:].rearrange("(d f) -> d f", d=D)    # [D, FF]
        wpjT_v = wpj_gath[:].rearrange("(f o) -> f o", f=FF)   # [FF, D]

        # ---------- Phase D: out-proj (fp8 DoubleRow) + residual + LN2 ----
        mT = main.tile([128, 16, 512], dt.float8e4, tag="bigB", bufs=2,
                       name="mT")
        h2_pool = tc.tile_pool(name="h2a", bufs=4)
        h2a = h2_pool.__enter__()
        h2acc = [h2a.tile([128, D], dt.float32, tag="h2", bufs=4,
                          name=f"h2_{t}") for t in range(4)]
        with tc.tile_pool(name="wo", bufs=3) as wop, \
             tc.tile_pool(name="zps", bufs=1, space="PSUM") as zps:
            for og2 in range(2):
                o_base = og2 * 1024
                pss = [None] * 8
                for g in range(8):          # dj pairs
                    wt = wop.tile([128, 2, 1024], dt.float8e4, tag="wot")
                    nc.sync.dma_start(
                        out=wt[:],
                        in_=bass.AP(tensor=wo_gath.tensor,
                                    offset=wo_gath.offset
                                    + 2 * g * 128 * D + o_base,
                                    ap=[[D, 128], [128 * D, 2], [1, 1024]]))
                    for osub in range(2):
                        for tb in range(4):
                            k = osub * 4 + tb
                            if pss[k] is None:
                                pss[k] = zps.tile([128, 512], dt.float32,
                                                  tag=f"z{k}", name=f"z_{k}")
                            nc.tensor.matmul(
                                pss[k][:],
                                OT[:, 2 * g:2 * g + 2,
                                   tb * 128:(tb + 1) * 128],
                                wt[:, :, osub * 512:(osub + 1) * 512],
                                start=(g == 0), stop=(g == 7),
                                perf_mode=PM.DoubleRow)
                for osub in range(2):
                    for tb in range(4):
                        sl = slice(o_base + osub * 512,
                                   o_base + osub * 512 + 512)
                        nc.vector.tensor_scalar_mul(h2acc[tb][:, sl],
                                                    pss[osub * 4 + tb][:],
                                                    1.0 / WSCALE)
                        nc.vector.tensor_add(h2acc[tb][:, sl],
                                             h2acc[tb][:, sl], bo_bc[:, sl])
        with tc.tile_pool(name="xd", bufs=2) as xd, \
             tc.tile_pool(name="md", bufs=1) as md, \
             tc.tile_pool(name="trps2", bufs=4, space="PSUM") as trps2:
            for tb in range(4):
                for xh in range(2):
                    x_t = xd.tile([128, 1024], dt.float32, tag="x2")
                    nc.sync.dma_start(
                        out=x_t[:],
                        in_=xl_d[tb * 128:(tb + 1) * 128,
                                 xh * 1024:(xh + 1) * 1024])
                    nc.vector.tensor_add(
                        h2acc[tb][:, xh * 1024:(xh + 1) * 1024],
                        h2acc[tb][:, xh * 1024:(xh + 1) * 1024], x_t[:])
                nc.sync.dma_start(out=h2_d[tb * 128:(tb + 1) * 128, :],
                                  in_=h2acc[tb][:])
                m_t = md.tile([128, D], dt.float32, tag="m")
                layernorm(h2acc[tb], m_t, "g2", "b2")
                for dj in range(16):
                    ps = trps2.tile([128, 128], dt.float32, tag="tp2")
                    nc.tensor.transpose(ps[:], m_t[:, dj * 128:(dj + 1) * 128],
                                        ident[:])
                    nc.vector.tensor_copy(mT[:, dj, tb * 128:(tb + 1) * 128],
                                          ps[:])

        h2_pool.__exit__(None, None, None)

        # ---------- Phase E: MLP (fp8 DoubleRow) ----------
        gt_pool = tc.tile_pool(name="gtpl", bufs=1)
        gtpl = gt_pool.__enter__()
        GT1 = gtpl.tile([128, 32, 512], dt.float8e4, name="GT1")
        GT2 = gtpl.tile([128, 32, 512], dt.float8e4, name="GT2")

        def gt_pair(k, c0, c1):
            # lhsT [128, 2, c1-c0] for fti pair (2k, 2k+1)
            if 2 * k < 32:
                return GT1[:, 2 * k:2 * k + 2, c0:c1]
            return GT2[:, 2 * k - 32:2 * k - 30, c0:c1]

        with tc.tile_pool(name="wfc", bufs=6) as wfcp, \
             tc.tile_pool(name="ups", bufs=1, space="PSUM") as ups:
            for FG in range(8):            # 1024 f-cols per group
                pss = [None] * 8
                for g in range(8):         # dj pairs
                    wt = wfcp.tile([128, 2, 1024], dt.float8e4, tag="wfct")
                    nc.sync.dma_start(
                        out=wt[:],
                        in_=bass.AP(tensor=wfc_gath.tensor,
                                    offset=wfc_gath.offset
                                    + 2 * g * 128 * FF + FG * 1024,
                                    ap=[[FF, 128], [128 * FF, 2], [1, 1024]]))
                    for fsub in range(8):
                        if pss[fsub] is None:
                            pss[fsub] = ups.tile([128, 512], dt.float32,
                                                 tag=f"u{fsub}",
                                                 name=f"u_{fsub}")
                        nc.tensor.matmul(
                            pss[fsub][:],
                            wt[:, :, fsub * 128:(fsub + 1) * 128],
                            mT[:, 2 * g:2 * g + 2, :],
                            start=(g == 0), stop=(g == 7),
                            perf_mode=PM.DoubleRow)
                for fsub in range(8):
                    fti = FG * 8 + fsub
                    dst = (GT1[:, fti, 0:512] if fti < 32
                           else GT2[:, fti - 32, 0:512])
                    nc.scalar.activation(dst, pss[fsub][:],
                                         AF.Gelu_apprx_tanh,
                                         bias=bfc_pp[:, fti:fti + 1],
                                         scale=1.0 / WSCALE)
        with tc.tile_pool(name="wpj", bufs=5) as wpjp, \
             tc.tile_pool(name="yps", bufs=1, space="PSUM") as yps, \
             tc.tile_pool(name="outp", bufs=6) as outp:
            for tg in range(2):            # tt groups of 2
                pss = {}
                h2s_g = {}
                for k in range(32):        # ft pairs
                    wt = wpjp.tile([128, 2, D], dt.float8e4, tag="wpjt")
                    nc.sync.dma_start(
                        out=wt[:],
                        in_=bass.AP(tensor=wpj_gath.tensor,
                                    offset=wpj_gath.offset + 2 * k * 128 * D,
                                    ap=[[D, 128], [128 * D, 2], [1, D]]))
                    for ob in range(4):
                        for ti in range(2):
                            tt = tg * 2 + ti
                            key = (ob, ti)
                            if key not in pss:
                                pss[key] = yps.tile(
                                    [128, 512], dt.float32,
                                    tag=f"y{ob}{ti}", name=f"y_{ob}_{ti}")
                            nc.tensor.matmul(
                                pss[key][:],
                                gt_pair(k, tt * 128, (tt + 1) * 128),
                                wt[:, :, ob * 512:(ob + 1) * 512],
                                start=(k == 0), stop=(k == 31),
                                perf_mode=PM.DoubleRow)
                for ti in range(2):
                    tt = tg * 2 + ti
                    h2s = outp.tile([128, D], dt.float32, tag="h2s",
                                    bufs=2, name=f"h2s_{tt}")
                    nc.sync.dma_start(
                        out=h2s[:], in_=h2_d[tt * 128:(tt + 1) * 128, :])
                    h2s_g[ti] = h2s
                for ob in range(4):
                    for ti in range(2):
                        tt = tg * 2 + ti
                        sl = slice(ob * 512, ob * 512 + 512)
                        o_t = outp.tile([128, 512], dt.float32, tag="o")
                        nc.vector.tensor_scalar_mul(o_t[:], pss[(ob, ti)][:],
                                                    1.0 / WSCALE)
                        nc.vector.tensor_add(o_t[:], o_t[:], bpj_bc[:, sl])
                        nc.vector.tensor_add(o_t[:], o_t[:],
                                             h2s_g[ti][:, sl])
                        nc.sync.dma_start(
                            out=out_d[tt * 128:(tt + 1) * 128, sl],
                            in_=o_t[:])
        gt_pool.__exit__(None, None, None)
        stack.close()

    nc.compile()
    return nc


def _host_prep(inputs):
    f32 = lambda k: np.ascontiguousarray(np.asarray(inputs[k], np.float32))
    x = f32("hidden_states")
    wqT = np.ascontiguousarray(f32("wq").T)
    wkT = np.ascontiguousarray(f32("wk").T)
    wvT = np.ascontiguousarray(f32("wv").T)
    woT = np.ascontiguousarray(f32("wo").T).ravel()
    wfcT = np.ascontiguousarray(f32("w_fc").T).ravel()
    wpjT = np.ascontiguousarray(f32("w_proj").T).ravel()
    kp = np.arange(128)
    qq = np.arange(128)
    in_maps = []
    for c in range(NC):
        mask = np.empty((128, 8, 128), np.float32)
        for j in range(8):
            mask[:, j, :] = np.where(
                8 * kp[:, None] + j <= 8 * qq[None, :] + c, 1.0, 0.0)
        in_maps.append({
            "xl": np.concatenate([x[0, c::NC, :], x[1, c::NC, :]], 0),
            "wqT": wqT, "wkT": wkT, "wvT": wvT,
            "wo_ch": woT[c * WO_CH:(c + 1) * WO_CH],
            "wfc_ch": wfcT[c * WFC_CH:(c + 1) * WFC_CH],
            "wpj_ch": wpjT[c * WPJ_CH:(c + 1) * WPJ_CH],
            "mask": mask.astype(ml_dtypes.bfloat16),
            "ln1g": f32("ln1_g"), "ln1b": f32("ln1_b"),
            "ln2g": f32("ln2_g"), "ln2b": f32("ln2_b"),
            "bo": f32("bo"), "bfc": f32("b_fc"), "bpj": f32("b_proj"),
        })
    return in_maps


def kernel(**inputs) -> np.ndarray:
    in_maps = _host_prep(inputs)
    key = (not bool(np.all(np.asarray(inputs["ln1_g"]) == 1.0)),
           not bool(np.all(np.asarray(inputs["ln1_b"]) == 0.0)),
           not bool(np.all(np.asarray(inputs["ln2_g"]) == 1.0)),
           not bool(np.all(np.asarray(inputs["ln2_b"]) == 0.0)))
    if key not in _CACHE:
        _CACHE[key] = _build(*key)
    nc = _CACHE[key]
    res = run_bass_kernel_spmd(nc, in_maps, core_ids=list(range(NC)))
    if res.exec_time_ns is not None:
        print(f"HW exec time: {res.exec_time_ns} ns")
    out = np.zeros((B, S, D), np.float32)
    for c in range(NC):
        o = res.results[c]["out"]
        out[0, c::NC] = o[:RPC]
        out[1, c::NC] = o[RPC:]
    return out


# revision 29
# speedup vs baseline: 1.2643x; 1.0096x over previous
"""BinaryGPTNeoBlock on 8 trn2 NeuronCores.

Sequence-parallel over 8 cores: core c owns rows {c, c+8, ...} of both
batch elements (256 per batch, 512 total). Collectives (one stream, in
program order): AllGather K (bf16), V (fp8), then tanh'd+scaled fp8
out-proj/fc/proj weights -- all triggered early so they hide under the
QKV projections and attention. Out-proj and both MLP matmuls run fp8
DoubleRow (weights scaled x1024 into e4m3 range, descaled on PSUM
read); q/k/scores stay bf16 for softmax fidelity. Attention exp reads
PSUM directly and causal masking is a multiplicative bf16 {0,1} mask
applied after exp.

Self-contained: hardcodes shapes; host only shards/transposes/builds masks.
"""

import numpy as np
import ml_dtypes

import concourse.bass as bass
import concourse.tile as tile
from concourse import bacc, mybir
from concourse.bass_utils import run_bass_kernel_spmd
from concourse.masks import make_identity

B, S, D = 2, 2048, 2048
H = 16
HD = 128
FF = 4 * D
EPS = 1e-5
NC = 8
RPC = S // NC          # 256 rows per core per batch
TL = 2 * RPC           # 512 local rows
NKV = TL * D           # elems of K^T (== of V) per core
WO_CH = D * D // NC
WFC_CH = D * FF // NC
WPJ_CH = FF * D // NC
WSCALE = 1024.0        # fp8 weight scale (w in +-0.0221 -> +-22.6)

dt = mybir.dt
AF = mybir.ActivationFunctionType
OP = mybir.AluOpType
PM = mybir.MatmulPerfMode

_CACHE = {}


def _build(apply_g1, apply_b1, apply_g2, apply_b2):
    nc = bacc.Bacc("TRN2", target_bir_lowering=False, debug=False,
                   num_devices=NC)

    xl_d = nc.dram_tensor("xl", [TL, D], dt.float32, kind="ExternalInput").ap()
    wqT_d = nc.dram_tensor("wqT", [D, D], dt.float32, kind="ExternalInput").ap()
    wkT_d = nc.dram_tensor("wkT", [D, D], dt.float32, kind="ExternalInput").ap()
    wvT_d = nc.dram_tensor("wvT", [D, D], dt.float32, kind="ExternalInput").ap()
    wo_ch_d = nc.dram_tensor("wo_ch", [WO_CH], dt.float32,
                             kind="ExternalInput").ap()
    wfc_ch_d = nc.dram_tensor("wfc_ch", [WFC_CH], dt.float32,
                              kind="ExternalInput").ap()
    wpj_ch_d = nc.dram_tensor("wpj_ch", [WPJ_CH], dt.float32,
                              kind="ExternalInput").ap()
    mask_d = nc.dram_tensor("mask", [128, 8, 128], dt.bfloat16,
                            kind="ExternalInput").ap()
    ln1g_d = nc.dram_tensor("ln1g", [D], dt.float32, kind="ExternalInput").ap()
    ln1b_d = nc.dram_tensor("ln1b", [D], dt.float32, kind="ExternalInput").ap()
    ln2g_d = nc.dram_tensor("ln2g", [D], dt.float32, kind="ExternalInput").ap()
    ln2b_d = nc.dram_tensor("ln2b", [D], dt.float32, kind="ExternalInput").ap()
    bo_d = nc.dram_tensor("bo", [D], dt.float32, kind="ExternalInput").ap()
    bfc_d = nc.dram_tensor("bfc", [FF], dt.float32, kind="ExternalInput").ap()
    bpj_d = nc.dram_tensor("bpj", [D], dt.float32, kind="ExternalInput").ap()
    out_d = nc.dram_tensor("out", [TL, D], dt.float32,
                           kind="ExternalOutput").ap()

    def bcast_row(src_ap, n):
        return bass.AP(tensor=src_ap.tensor, offset=src_ap.offset,
                       ap=[[0, 128], [1, n]])

    with tile.TileContext(nc) as tc:
        import contextlib
        stack = contextlib.ExitStack()
        main = stack.enter_context(tc.tile_pool(name="main", bufs=1))
        dram = stack.enter_context(
            tc.tile_pool(name="dram", bufs=1, space="DRAM"))

        ident = main.tile([128, 128], dt.float32)
        make_identity(nc, ident[:])
        ones_col = main.tile([128, 1], dt.float32)
        nc.vector.memset(ones_col[:], 1.0)
        ones_col_b = main.tile([128, 1], dt.bfloat16)
        nc.vector.tensor_copy(ones_col_b[:], ones_col[:])
        ones_row = main.tile([1, 128], dt.float32)
        nc.vector.memset(ones_row[:], 1.0)
        eps_t = main.tile([128, 1], dt.float32)
        nc.vector.memset(eps_t[:], EPS)
        bo_bc = main.tile([128, D], dt.float32)
        nc.sync.dma_start(out=bo_bc[:], in_=bcast_row(bo_d, D))
        bpj_bc = main.tile([128, D], dt.float32)
        nc.sync.dma_start(out=bpj_bc[:], in_=bcast_row(bpj_d, D))
        ln_bc = {}
        for nm, flag, src in (("g1", apply_g1, ln1g_d),
                              ("b1", apply_b1, ln1b_d),
                              ("g2", apply_g2, ln2g_d),
                              ("b2", apply_b2, ln2b_d)):
            if flag:
                t = main.tile([128, D], dt.float32, name=f"ln_{nm}")
                nc.sync.dma_start(out=t[:], in_=bcast_row(src, D))
                ln_bc[nm] = t
        bfc_pp = main.tile([128, FF // 128], dt.float32)
        nc.sync.dma_start(
            out=bfc_pp[:],
            in_=bass.AP(tensor=bfc_d.tensor, offset=bfc_d.offset,
                        ap=[[1, 128], [128, FF // 128]]))
        mask_pool = tc.tile_pool(name="maskp", bufs=1)
        maskp = mask_pool.__enter__()
        masks = maskp.tile([128, 8, 128], dt.bfloat16)
        nc.sync.dma_start(out=masks[:], in_=mask_d[:])

        # Long-lived K/V staging pool (created before phase-B pools so its
        # SBUF doesn't alias them; sized for 2 head-groups of prefetch).
        kvh_pool = tc.tile_pool(name="kvh", bufs=1)
        kvh = kvh_pool.__enter__()

        # big rotating slots: hT -> OT reuse, QT -> mT reuse
        hT = main.tile([128, 16, 512], dt.bfloat16, tag="bigA", bufs=2,
                       name="hT")
        QT = main.tile([128, 16, 512], dt.bfloat16, tag="bigA", bufs=2,
                       name="QT")

        h2_d = dram.tile([TL, D], dt.float32)

        def layernorm(x_t, h_t, gk, bk):
            with tc.tile_pool(name="lnp", bufs=2) as lp:
                st = lp.tile([128, 4, 6], dt.float32, tag="st")
                xr = x_t[:].rearrange("p (n f) -> p n f", n=4)
                for sg in range(4):
                    nc.vector.bn_stats(out=st[:, sg, :], in_=xr[:, sg, :])
                mv = lp.tile([128, 2], dt.float32, tag="mv")
                nc.vector.bn_aggr(out=mv[:], in_=st[:])
                std = lp.tile([128, 1], dt.float32, tag="sd")
                nc.scalar.activation(std[:], mv[:, 1:2], AF.Sqrt,
                                     bias=eps_t[:])
                rstd = lp.tile([128, 1], dt.float32, tag="rs")
                nc.vector.reciprocal(rstd[:], std[:])
                nc.vector.tensor_scalar(h_t[:], x_t[:], mv[:, 0:1], rstd[:],
                                        op0=OP.subtract, op1=OP.mult)
                if gk in ln_bc:
                    nc.vector.tensor_mul(h_t[:], h_t[:], ln_bc[gk][:])
                if bk in ln_bc:
                    nc.vector.tensor_add(h_t[:], h_t[:], ln_bc[bk][:])

        # ---------- Phase A: x -> LN1 -> h^T ----------
        with tc.tile_pool(name="xa", bufs=2) as xa, \
             tc.tile_pool(name="ha", bufs=2) as ha, \
             tc.tile_pool(name="trps", bufs=4, space="PSUM") as trps:
            for tb in range(4):
                x_t = xa.tile([128, D], dt.float32, tag="x")
                nc.sync.dma_start(out=x_t[:],
                                  in_=xl_d[tb * 128:(tb + 1) * 128, :])
                h_t = ha.tile([128, D], dt.float32, tag="h")
                layernorm(x_t, h_t, "g1", "b1")
                for dj in range(16):
                    ps = trps.tile([128, 128], dt.float32, tag="tp")
                    nc.tensor.transpose(ps[:], h_t[:, dj * 128:(dj + 1) * 128],
                                        ident[:])
                    nc.vector.tensor_copy(hT[:, dj, tb * 128:(tb + 1) * 128],
                                          ps[:])

        # wprep: tanh+scale fp8 chunks of wo/wfc/wpj, emitted interleaved
        # into the attention head-group loop so they fill ACT/DMA idle
        # slots there (program-order priority puts them behind attention).
        wo_bounce = dram.tile([WO_CH], dt.float8e4)
        wfc_bounce = dram.tile([WFC_CH], dt.float8e4)
        wpj_bounce = dram.tile([WPJ_CH], dt.float8e4)
        wprep_pool = tc.tile_pool(name="wprep", bufs=2)
        wprep = wprep_pool.__enter__()
        _wprep_jobs = []
        _wprep_gated = []
        for src, dst, n_t in ((wo_ch_d, wo_bounce, WO_CH // (128 * 2048)),
                              (wfc_ch_d, wfc_bounce, WFC_CH // (128 * 2048)),
                              (wpj_ch_d, wpj_bounce, WPJ_CH // (128 * 2048))):
            for i in range(n_t - 1):
                _wprep_jobs.append((src, dst, i, False))
            _wprep_gated.append((src, dst, n_t - 1, True))

        # gate tile: exactly 1.0, but data-dependent on the V gather output.
        # Multiplying the FINAL chunk of each fp8 weight bounce by it makes
        # the wo/wfc/wpj AllGather triggers depend on the V gather, pinning
        # the serial collective queue to the order K, V, wo, wfc, wpj.
        gate = [None]

        def emit_wprep(n, jobs=None):
            jobs = _wprep_jobs if jobs is None else jobs
            for _ in range(n):
                if not jobs:
                    return
                src, dst, i, is_last = jobs.pop(0)
                raw = wprep.tile([128, 2048], dt.float32, tag="wraw")
                nc.sync.dma_start(
                    out=raw[:],
                    in_=src[i * 128 * 2048:(i + 1) * 128 * 2048]
                    .rearrange("(p f) -> p f", p=128))
                tnh = wprep.tile([128, 2048], dt.bfloat16, tag="wtnh")
                nc.scalar.activation(tnh[:], raw[:], AF.Tanh)
                sc8 = wprep.tile([128, 2048], dt.float8e4, tag="wsc")
                if is_last:
                    nc.vector.tensor_scalar(sc8[:], tnh[:], gate[0][:],
                                            WSCALE, op0=OP.mult, op1=OP.mult)
                else:
                    nc.vector.tensor_scalar_mul(sc8[:], tnh[:], WSCALE)
                nc.sync.dma_start(
                    out=dst[i * 128 * 2048:(i + 1) * 128 * 2048]
                    .rearrange("(p f) -> p f", p=128), in_=sc8[:])

        # all tanh work happens BEFORE the first attention exp: the ACT
        # engine's activation-table reload (~2.7us per function switch)
        # makes interleaving tanh with exp ruinous.
        emit_wprep(len(_wprep_jobs))

        # ---------- Phase B: QKV ----------
        # K/V bounces and gathers are split into feature halves (A: head
        # groups 0-1, B: 2-3) so each half's AllGather pipelines with the
        # projection of the other half and attention can start on half A.
        HKV = NKV // 2
        k_bounce = [dram.tile([HKV], dt.bfloat16, name=f"kb{i}")
                    for i in range(2)]
        v_bounce = [dram.tile([HKV], dt.float8e4, name=f"vb{i}")
                    for i in range(2)]
        k_gath = [dram.tile([NC * HKV], dt.bfloat16, addr_space="Shared",
                            name=f"kg{i}") for i in range(2)]
        v_gath = [dram.tile([NC * HKV], dt.float8e4, addr_space="Shared",
                            name=f"vg{i}") for i in range(2)]
        wo_gath = dram.tile([NC * WO_CH], dt.float8e4, addr_space="Shared")
        wfc_gath = dram.tile([NC * WFC_CH], dt.float8e4, addr_space="Shared")
        wpj_gath = dram.tile([NC * WPJ_CH], dt.float8e4, addr_space="Shared")

        def project_qk(wT_dram, kind):
            # feature-major output via PE transpose; og(4) x [128,512] loads
            with tc.tile_pool(name=f"pw_{kind}", bufs=4) as wp, \
                 tc.tile_pool(name=f"po_{kind}", bufs=4) as op_, \
                 tc.tile_pool(name=f"pp_{kind}", bufs=1, space="PSUM") as pp, \
                 tc.tile_pool(name=f"pt_{kind}", bufs=4, space="PSUM") as tp2:
                for og in range(4):
                    o_base = og * 512
                    ktacc = []
                    if kind == "k":
                        for k4 in range(4):
                            ka = op_.tile([128, 512], dt.bfloat16, tag="ka",
                                          bufs=8, name=f"ka_{og}_{k4}")
                            ktacc.append(ka)
                    pss = [None] * 4
                    for dj in range(16):
                        raw = wp.tile([128, 512], dt.float32, tag="raw")
                        nc.sync.dma_start(
                            out=raw[:],
                            in_=wT_dram[dj * 128:(dj + 1) * 128,
                                        o_base:o_base + 512])
                        tnh = wp.tile([128, 512], dt.bfloat16, tag="tnh")
                        nc.scalar.activation(tnh[:], raw[:], AF.Tanh)
                        for tb in range(4):
                            if pss[tb] is None:
                                pss[tb] = pp.tile([128, 512], dt.float32,
                                                  tag=f"ps{tb}",
                                                  name=f"ps_{kind}_{tb}")
                            nc.tensor.matmul(
                                pss[tb][:],
                                hT[:, dj, tb * 128:(tb + 1) * 128],
                                tnh[:], start=(dj == 0), stop=(dj == 15))
                    for tb in range(4):
                        tm = op_.tile([128, 512], dt.float32, tag="tm")
                        nc.vector.tensor_copy(tm[:], pss[tb][:])
                        for k4 in range(4):
                            dj2 = (o_base + k4 * 128) // 128
                            ps2 = tp2.tile([128, 128], dt.float32, tag="t2")
                            nc.tensor.transpose(
                                ps2[:], tm[:, k4 * 128:(k4 + 1) * 128],
                                ident[:])
                            if kind == "q":
                                nc.vector.tensor_copy(
                                    QT[:, dj2, tb * 128:(tb + 1) * 128],
                                    ps2[:])
                            else:
                                nc.vector.tensor_copy(
                                    ktacc[k4][:, tb * 128:(tb + 1) * 128],
                                    ps2[:])
                    if kind == "k":
                        for k4 in range(4):
                            dj2 = (o_base + k4 * 128) // 128
                            half, dj2h = dj2 // 8, dj2 % 8
                            nc.sync.dma_start(
                                out=k_bounce[half][dj2h * 128 * TL:
                                                   (dj2h + 1) * 128 * TL]
                                .rearrange("(p t) -> p t", p=128),
                                in_=ktacc[k4][:])
                    if kind == "k" and og == 1:
                        nc.gpsimd.collective_compute(
                            "AllGather", OP.bypass,
                            replica_groups=[list(range(NC))],
                            ins=[k_bounce[0][:]], outs=[k_gath[0][:]])
                    if kind == "k" and og == 3:
                        nc.gpsimd.collective_compute(
                            "AllGather", OP.bypass,
                            replica_groups=[list(range(NC))],
                            ins=[k_bounce[1][:]], outs=[k_gath[1][:]])

        def project_v(wT_dram):
            # token-major; og2(2) x [128,1024] loads; per-half fp8 stores
            # + per-half AllGather right after each feature half finishes
            with tc.tile_pool(name="pw_v", bufs=4) as wp, \
                 tc.tile_pool(name="po_v", bufs=4) as op_, \
                 tc.tile_pool(name="pp_v", bufs=1, space="PSUM") as pp:
                for og2 in range(2):
                    o_base = og2 * 1024
                    vacc = [op_.tile([128, 1024], dt.float8e4, tag="va",
                                     bufs=8, name=f"va_{og2}_{t}")
                            for t in range(4)]
                    pss = [None] * 8
                    for dj in range(16):
                        raw = wp.tile([128, 1024], dt.float32, tag="raw")
                        nc.sync.dma_start(
                            out=raw[:],
                            in_=wT_dram[dj * 128:(dj + 1) * 128,
                                        o_base:o_base + 1024])
                        tnh = wp.tile([128, 1024], dt.bfloat16, tag="tnh")
                        nc.scalar.activation(tnh[:], raw[:], AF.Tanh)
                        for osub in range(2):
                            for tb in range(4):
                                k = osub * 4 + tb
                                if pss[k] is None:
                                    pss[k] = pp.tile([128, 512], dt.float32,
                                                     tag=f"ps{k}",
                                                     name=f"ps_v_{k}")
                                nc.tensor.matmul(
                                    pss[k][:],
                                    hT[:, dj, tb * 128:(tb + 1) * 128],
                                    tnh[:, osub * 512:(osub + 1) * 512],
                                    start=(dj == 0), stop=(dj == 15))
                    for osub in range(2):
                        for tb in range(4):
                            sl = slice(osub * 512, osub * 512 + 512)
                            nc.vector.tensor_copy(vacc[tb][:, sl],
                                                  pss[osub * 4 + tb][:])
                    for tb in range(4):
                        nc.sync.dma_start(
                            out=v_bounce[og2][tb * 128 * 1024:
                                              (tb + 1) * 128 * 1024]
                            .rearrange("(p t) -> p t", p=128),
                            in_=vacc[tb][:])
                    nc.gpsimd.collective_compute(
                        "AllGather", OP.bypass,
                        replica_groups=[list(range(NC))],
                        ins=[v_bounce[og2][:]], outs=[v_gath[og2][:]])

        project_qk(wkT_d, "k")
        project_v(wvT_d)
        gate_src = main.tile([128, 1], dt.float8e4, name="gate_src")
        nc.sync.dma_start(
            out=gate_src[:],
            in_=bass.AP(tensor=v_gath[1].tensor, offset=v_gath[1].offset,
                        ap=[[1, 128], [1, 1]]))
        gate_t = main.tile([128, 1], dt.float32, name="gate_t")
        nc.vector.tensor_scalar(gate_t[:], gate_src[:], 0.0, 1.0,
                                op0=OP.mult, op1=OP.add)
        gate[0] = gate_t
        emit_wprep(3, _wprep_gated)
        nc.gpsimd.collective_compute(
            "AllGather", OP.bypass, replica_groups=[list(range(NC))],
            ins=[wo_bounce[:]], outs=[wo_gath[:]])
        nc.gpsimd.collective_compute(
            "AllGather", OP.bypass, replica_groups=[list(range(NC))],
            ins=[wfc_bounce[:]], outs=[wfc_gath[:]])
        nc.gpsimd.collective_compute(
            "AllGather", OP.bypass, replica_groups=[list(range(NC))],
            ins=[wpj_bounce[:]], outs=[wpj_gath[:]])
        project_qk(wqT_d, "q")

        # ---------- Phase C: attention ----------
        # Causal structure (strided sharding, chunk = 128 partitions of
        # k-tokens): key-half tb=0 (global k < 1024) is fully unmasked for
        # local q-half 1 (global q >= 1024) and diagonal for q-half 0;
        # key-half tb=1 is fully masked for q-half 0 (skipped) and diagonal
        # for q-half 1. The diagonal 128x128 mask is the SAME bf16 {0,1}
        # tile for both halves. exp runs on wide tiles to amortize the
        # ACT per-instruction overhead.
        OT = main.tile([128, 16, 512], dt.float8e4, tag="bigB", bufs=2,
                       name="OT")
        with tc.tile_pool(name="att", bufs=4) as att, \
             tc.tile_pool(name="attsm", bufs=4) as attsm, \
             tc.tile_pool(name="stps", bufs=2, space="PSUM") as stps, \
             tc.tile_pool(name="otps", bufs=2, space="PSUM") as otps, \
             tc.tile_pool(name="dnps", bufs=1, space="PSUM") as dnps, \
             tc.tile_pool(name="bcps", bufs=1, space="PSUM") as bcps:
            for hg in range(4):            # head groups of 4
                kg, vg_ = k_gath[hg // 2], v_gath[hg // 2]
                hgh = hg % 2
                kt_g, v_g = [], []
                for j in range(NC):
                    kt = kvh.tile([128, 4, 512], dt.bfloat16, tag="kth",
                                  bufs=12, name=f"kt_{hg}_{j}")
                    nc.sync.dma_start(
                        out=kt[:],
                        in_=bass.AP(tensor=kg.tensor,
                                    offset=kg.offset + j * HKV
                                    + hgh * 4 * 128 * TL,
                                    ap=[[TL, 128], [128 * TL, 4], [1, TL]]))
                    kt_g.append(kt)
                    vt = kvh.tile([128, 4, 512], dt.float8e4, tag="vth",
                                  bufs=12, name=f"vt_{hg}_{j}")
                    nc.sync.dma_start(
                        out=vt[:],
                        in_=bass.AP(tensor=vg_.tensor,
                                    offset=vg_.offset + j * HKV
                                    + hgh * 4 * 128,
                                    ap=[[1024, 128], [128 * 1024, 4],
                                        [1, 512]]))
                    v_g.append(vt)
                for hh in range(4):
                    h = hg * 4 + hh
                    # both batches accumulate into one shared PSUM pair
                    # (disjoint column ranges; per-element has_written
                    # handles the staggered starts) for a deeper
                    # independent MM pipeline per head.
                    ot_ps = otps.tile([128, 512], dt.float32, tag="ot")
                    dn_ps = dnps.tile([1, 512], dt.float32, tag="dn")
                    first = True
                    for b in range(2):
                        for tb in range(2):
                            for jq in range(2):    # chunk quads
                                st = stps.tile([128, 1024], dt.float32,
                                               tag="st")
                                pt_e = attsm.tile([128, 1024], dt.bfloat16,
                                                  tag="pte")
                                st3 = st[:].rearrange(
                                    "p (c q) -> p c q", c=4)
                                pe3 = pt_e[:].rearrange(
                                    "p (c q) -> p c q", c=4)
                                for jj in range(4):
                                    j = 4 * jq + jj
                                    if tb == 0:
                                        nc.tensor.matmul(
                                            st[:, jj * 256:jj * 256 + 256],
                                            kt_g[j][:, hh,
                                                    b * 256:b * 256 + 128],
                                            QT[:, h, b * 256:(b + 1) * 256],
                                            start=True, stop=True)
                                    else:
                                        nc.tensor.matmul(
                                            st[:, jj * 256 + 128:
                                               jj * 256 + 256],
                                            kt_g[j][:, hh, b * 256 + 128:
                                                    b * 256 + 256],
                                            QT[:, h, b * 256 + 128:
                                               b * 256 + 256],
                                            start=True, stop=True)
                                if tb == 0:
                                    nc.scalar.activation(pt_e[:], st[:],
                                                         AF.Exp)
                                    # diagonal mask on q-half 0 quarters
                                    nc.vector.tensor_mul(
                                        pe3[:, :, 0:128], pe3[:, :, 0:128],
                                        masks[:, 4 * jq:4 * jq + 4, :])
                                else:
                                    nc.scalar.activation(
                                        pe3[:, :, 128:256],
                                        st3[:, :, 128:256], AF.Exp)
                                    nc.vector.tensor_mul(
                                        pe3[:, :, 128:256],
                                        pe3[:, :, 128:256],
                                        masks[:, 4 * jq:4 * jq + 4, :])
                                for jj in range(4):
                                    j = 4 * jq + jj
                                    last = (b == 1 and tb == 1 and jq == 1
                                            and jj == 3)
                                    if tb == 0:
                                        rhs = pt_e[:, jj * 256:jj * 256 + 256]
                                        ot_dst = ot_ps[:, b * 256:
                                                       b * 256 + 256]
                                        dn_dst = dn_ps[:, b * 256:
                                                       b * 256 + 256]
                                    else:
                                        rhs = pt_e[:, jj * 256 + 128:
                                                   jj * 256 + 256]
                                        ot_dst = ot_ps[:, b * 256 + 128:
                                                       b * 256 + 256]
                                        dn_dst = dn_ps[:, b * 256 + 128:
                                                       b * 256 + 256]
                                    nc.tensor.matmul(
                                        ot_dst,
                                        v_g[j][:, 2 * b + tb,
                                               hh * 128:(hh + 1) * 128],
                                        rhs, start=first, stop=last,
                                        skip_group_check=True)
                                    nc.tensor.matmul(
                                        dn_dst, ones_col_b[:], rhs,
                                        start=first, stop=last,
                                        skip_group_check=True)
                                    first = False
                    dn_sb = att.tile([1, 512], dt.float32, tag="dns")
                    nc.vector.reciprocal(dn_sb[:], dn_ps[:])
                    bc_ps = bcps.tile([128, 512], dt.float32, tag="bc")
                    nc.tensor.matmul(bc_ps[:], ones_row[:], dn_sb[:],
                                     start=True, stop=True)
                    bc_sb = att.tile([128, 512], dt.float32, tag="bcs")
                    nc.vector.tensor_copy(bc_sb[:], bc_ps[:])
                    nc.vector.tensor_mul(OT[:, h, :], ot_ps[:], bc_sb[:])
        wprep_pool.__exit__(None, None, None)
        kvh_pool.__exit__(None, None, None)
        mask_pool.__exit__(None, None, None)

        wo_v = wo_gath[:].rearrange("(d o) -> d o", d=D)       # [D, D]
        wfcT_v = wfc_gath[:].rearrange("(d f) -> d f", d=D)    # [D, FF]
        wpjT_v = wpj_gath[:].rearrange("(f o) -> f o", f=FF)   # [FF, D]

        # ---------- Phase D: out-proj (fp8 DoubleRow) + residual + LN2 ----
        mT = main.tile([128, 16, 512], dt.float8e4, tag="bigB", bufs=2,
                       name="mT")
        h2_pool = tc.tile_pool(name="h2a", bufs=4)
        h2a = h2_pool.__enter__()
        h2acc = [h2a.tile([128, D], dt.float32, tag="h2", bufs=4,
                          name=f"h2_{t}") for t in range(4)]
        with tc.tile_pool(name="wo", bufs=3) as wop, \
             tc.tile_pool(name="zps", bufs=1, space="PSUM") as zps:
            for og2 in range(2):
                o_base = og2 * 1024
                pss = [None] * 8
                for g in range(8):          # dj pairs
                    wt = wop.tile([128, 2, 1024], dt.float8e4, tag="wot")
                    nc.sync.dma_start(
                        out=wt[:],
                        in_=bass.AP(tensor=wo_gath.tensor,
                                    offset=wo_gath.offset
                                    + 2 * g * 128 * D + o_base,
                                    ap=[[D, 128], [128 * D, 2], [1, 1024]]))
                    for osub in range(2):
                        for tb in range(4):
                            k = osub * 4 + tb
                            if pss[k] is None:
                                pss[k] = zps.tile([128, 512], dt.float32,
                                                  tag=f"z{k}", name=f"z_{k}")
                            nc.tensor.matmul(
                                pss[k][:],
                                OT[:, 2 * g:2 * g + 2,
                                   tb * 128:(tb + 1) * 128],
                                wt[:, :, osub * 512:(osub + 1) * 512],
                                start=(g == 0), stop=(g == 7),
                                perf_mode=PM.DoubleRow)
                for osub in range(2):
                    for tb in range(4):
                        sl = slice(o_base + osub * 512,
                                   o_base + osub * 512 + 512)
                        nc.vector.tensor_scalar_mul(h2acc[tb][:, sl],
                                                    pss[osub * 4 + tb][:],
                                                    1.0 / WSCALE)
                        nc.vector.tensor_add(h2acc[tb][:, sl],
                                             h2acc[tb][:, sl], bo_bc[:, sl])
        with tc.tile_pool(name="xd", bufs=2) as xd, \
             tc.tile_pool(name="md", bufs=1) as md, \
             tc.tile_pool(name="trps2", bufs=4, space="PSUM") as trps2:
            for tb in range(4):
                for xh in range(2):
                    x_t = xd.tile([128, 1024], dt.float32, tag="x2")
                    nc.sync.dma_start(
                        out=x_t[:],
                        in_=xl_d[tb * 128:(tb + 1) * 128,
                                 xh * 1024:(xh + 1) * 1024])
                    nc.vector.tensor_add(
                        h2acc[tb][:, xh * 1024:(xh + 1) * 1024],
                        h2acc[tb][:, xh * 1024:(xh + 1) * 1024], x_t[:])
                nc.sync.dma_start(out=h2_d[tb * 128:(tb + 1) * 128, :],
                                  in_=h2acc[tb][:])
                m_t = md.tile([128, D], dt.float32, tag="m")
                layernorm(h2acc[tb], m_t, "g2", "b2")
                for dj in range(16):
                    ps = trps2.tile([128, 128], dt.float32, tag="tp2")
                    nc.tensor.transpose(ps[:], m_t[:, dj * 128:(dj + 1) * 128],
                                        ident[:])
                    nc.vector.tensor_copy(mT[:, dj, tb * 128:(tb + 1) * 128],
                                          ps[:])

        h2_pool.__exit__(None, None, None)

        # ---------- Phase E: MLP (fp8 DoubleRow) ----------
        gt_pool = tc.tile_pool(name="gtpl", bufs=1)
        gtpl = gt_pool.__enter__()
        GT1 = gtpl.tile([128, 32, 512], dt.float8e4, name="GT1")
        GT2 = gtpl.tile([128, 32, 512], dt.float8e4, name="GT2")

        def gt_pair(k, c0, c1):
            # lhsT [128, 2, c1-c0] for fti pair (2k, 2k+1)
            if 2 * k < 32:
                return GT1[:, 2 * k:2 * k + 2, c0:c1]
            return GT2[:, 2 * k - 32:2 * k - 30, c0:c1]

        with tc.tile_pool(name="wfc", bufs=6) as wfcp, \
             tc.tile_pool(name="ups", bufs=1, space="PSUM") as ups:
            for FG in range(8):            # 1024 f-cols per group
                pss = [None] * 8
                for g in range(8):         # dj pairs
                    wt = wfcp.tile([128, 2, 1024], dt.float8e4, tag="wfct")
                    nc.sync.dma_start(
                        out=wt[:],
                        in_=bass.AP(tensor=wfc_gath.tensor,
                                    offset=wfc_gath.offset
                                    + 2 * g * 128 * FF + FG * 1024,
                                    ap=[[FF, 128], [128 * FF, 2], [1, 1024]]))
                    for fsub in range(8):
                        if pss[fsub] is None:
                            pss[fsub] = ups.tile([128, 512], dt.float32,
                                                 tag=f"u{fsub}",
                                                 name=f"u_{fsub}")
                        nc.tensor.matmul(
                            pss[fsub][:],
                            wt[:, :, fsub * 128:(fsub + 1) * 128],
                            mT[:, 2 * g:2 * g + 2, :],
                            start=(g == 0), stop=(g == 7),
                            perf_mode=PM.DoubleRow)
                for fsub in range(8):
                    fti = FG * 8 + fsub
                    dst = (GT1[:, fti, 0:512] if fti < 32
                           else GT2[:, fti - 32, 0:512])
                    nc.scalar.activation(dst, pss[fsub][:],
                                         AF.Gelu_apprx_tanh,
                                         bias=bfc_pp[:, fti:fti + 1],
                                         scale=1.0 / WSCALE)
        with tc.tile_pool(name="wpj", bufs=5) as wpjp, \
             tc.tile_pool(name="yps", bufs=1, space="PSUM") as yps, \
             tc.tile_pool(name="outp", bufs=6) as outp:
            for tg in range(2):            # tt groups of 2
                pss = {}
                h2s_g = {}
                for k in range(32):        # ft pairs
                    wt = wpjp.tile([128, 2, D], dt.float8e4, tag="wpjt")
                    nc.sync.dma_start(
                        out=wt[:],
                        in_=bass.AP(tensor=wpj_gath.tensor,
                                    offset=wpj_gath.offset + 2 * k * 128 * D,
                                    ap=[[D, 128], [128 * D, 2], [1, D]]))
                    for ob in range(4):
                        for ti in range(2):
                            tt = tg * 2 + ti
                            key = (ob, ti)
                            if key not in pss:
                                pss[key] = yps.tile(
                                    [128, 512], dt.float32,
                                    tag=f"y{ob}{ti}", name=f"y_{ob}_{ti}")
                            nc.tensor.matmul(
                                pss[key][:],
                                gt_pair(k, tt * 128, (tt + 1) * 128),
                                wt[:, :, ob * 512:(ob + 1) * 512],
                                start=(k == 0), stop=(k == 31),
                                perf_mode=PM.DoubleRow)
                for ti in range(2):
                    tt = tg * 2 + ti
                    h2s = outp.tile([128, D], dt.float32, tag="h2s",
                                    bufs=2, name=f"h2s_{tt}")
                    nc.sync.dma_start(
                        out=h2s[:], in_=h2_d[tt * 128:(tt + 1) * 128, :])
                    h2s_g[ti] = h2s
                for ob in range(4):
                    for ti in range(2):
                        tt = tg * 2 + ti
                        sl = slice(ob * 512, ob * 512 + 512)
                        o_t = outp.tile([128, 512], dt.float32, tag="o")
                        nc.vector.tensor_scalar_mul(o_t[:], pss[(ob, ti)][:],
                                                    1.0 / WSCALE)
                        nc.vector.tensor_add(o_t[:], o_t[:], bpj_bc[:, sl])
                        nc.vector.tensor_add(o_t[:], o_t[:],
                                             h2s_g[ti][:, sl])
                        nc.sync.dma_start(
                            out=out_d[tt * 128:(tt + 1) * 128, sl],
                            in_=o_t[:])
        gt_pool.__exit__(None, None, None)
        stack.close()

    nc.compile()
    return nc


def _host_prep(inputs):
    f32 = lambda k: np.ascontiguousarray(np.asarray(inputs[k], np.float32))
    x = f32("hidden_states")
    wqT = np.ascontiguousarray(f32("wq").T)
    wkT = np.ascontiguousarray(f32("wk").T)
    wvT = np.ascontiguousarray(f32("wv").T)
    woT = np.ascontiguousarray(f32("wo").T).ravel()
    wfcT = np.ascontiguousarray(f32("w_fc").T).ravel()
    wpjT = np.ascontiguousarray(f32("w_proj").T).ravel()
    kp = np.arange(128)
    qq = np.arange(128)
    in_maps = []
    for c in range(NC):
        mask = np.empty((128, 8, 128), np.float32)
        for j in range(8):
            mask[:, j, :] = np.where(
                8 * kp[:, None] + j <= 8 * qq[None, :] + c, 1.0, 0.0)
        in_maps.append({
            "xl": np.concatenate([x[0, c::NC, :], x[1, c::NC, :]], 0),
            "wqT": wqT, "wkT": wkT, "wvT": wvT,
            "wo_ch": woT[c * WO_CH:(c + 1) * WO_CH],
            "wfc_ch": wfcT[c * WFC_CH:(c + 1) * WFC_CH],
            "wpj_ch": wpjT[c * WPJ_CH:(c + 1) * WPJ_CH],
            "mask": mask.astype(ml_dtypes.bfloat16),
            "ln1g": f32("ln1_g"), "ln1b": f32("ln1_b"),
            "ln2g": f32("ln2_g"), "ln2b": f32("ln2_b"),
            "bo": f32("bo"), "bfc": f32("b_fc"), "bpj": f32("b_proj"),
        })
    return in_maps


def kernel(**inputs) -> np.ndarray:
    in_maps = _host_prep(inputs)
    key = (not bool(np.all(np.asarray(inputs["ln1_g"]) == 1.0)),
           not bool(np.all(np.asarray(inputs["ln1_b"]) == 0.0)),
           not bool(np.all(np.asarray(inputs["ln2_g"]) == 1.0)),
           not bool(np.all(np.asarray(inputs["ln2_b"]) == 0.0)))
    if key not in _CACHE:
        _CACHE[key] = _build(*key)
    nc = _CACHE[key]
    res = run_bass_kernel_spmd(nc, in_maps, core_ids=list(range(NC)))
    if res.exec_time_ns is not None:
        print(f"HW exec time: {res.exec_time_ns} ns")
    out = np.zeros((B, S, D), np.float32)
    for c in range(NC):
        o = res.results[c]["out"]
        out[0, c::NC] = o[:RPC]
        out[1, c::NC] = o[RPC:]
    return out
